# revision 29
# baseline (speedup 1.0000x reference)
"""LSTM (single layer, final hidden state) on 8 Trainium2 NeuronCores.

Reference computation (per batch row b):
    gx[t] = x[t] @ w_ih.T + (b_ih + b_hh)
    g     = gx[t] + h @ w_hh.T          # [B, 4H], gate order i,f,g,o
    i,f,o = sigmoid(...), g_c = tanh(...)
    c     = f*c + i*g_c
    h     = o * tanh(c)
returns h after T steps, shape [1, B, H].

Sharding: data-parallel over batch B=256 -> 8 cores x 32. Weights replicated.

Per-core layout ("packed"): partition p = 32*j + b, where j in [0,4) indexes
an H-quarter (H index = 64*j + s, s in [0,64)) and b in [0,32) is the local
batch.  All elementwise tiles are [128, *].

The wall time is T x the per-step dependency-chain latency.  On top of the
previous all-matmul/packing structure, this version crushes the ACT
(scalar-engine) spine cost: every ACT instruction costs ~N*0.83ns + ~150ns
fixed (SBUF/PSUM access pipe), so the 4 activation calls/step of the old
kernel (~1.4us of ACT busy, serialized on the spine) dominate.  Changes:
  * tanh(g) = 2*sigmoid(2g) - 1 with the 2x folded into the g rows of
    w_ih/w_hh/bias host-side -> ALL gates go through ONE Sigmoid call.
  * Gates grouped (f,g,i | o) in TWO psum tiles: sigmoid(f,g,i) [N=192]
    fires as soon as the fgi h-rounds stop (o-rounds still streaming);
    sigmoid(o) [N=64] runs off the c-spine in ACT's idle window.
  * c-chain on DVE: gg=2*sg-1 (tensor_scalar, 4x mode), u=gg*i,
    cf=f*c, c=cf+u; then tanh(c) [N=64] and h=o*tanh(c), hT transpose.
  * Gate tiles in fp16 (not bf16): kills the 2sg-1 cancellation error and
    halves DVE read traffic same as bf16; h stays bf16 for the PE lhsT.
  * c state in fp32 SBUF (DVE access 58cyc vs 120 psum; ACT tanh pays
    +50cyc reading SBUF but the DVE chain saves more).

Everything h-independent stays off the spine as before: bias (two bf16
rounds reproducing the fp32 bias), x rounds for step t+1 opened during step
t's elementwise window, x chunks DMA-prefetched + PE-transposed, and a
zero-contribution warm-up matmul chained mid-window keeps the PE p-state
at 2.4GHz.
"""

import os
import sys

import numpy as np

B_TOT, T_FULL, I_DIM, H = 256, 1024, 128, 256
NCORES = 8
B = B_TOT // NCORES  # 32 per core
NJ = 4  # H quarters
S = H // NJ  # 64
# (row base in the PyTorch i,f,g,o layout, pre-scale) per column block
FGI_BLOCKS = ((256, 1.0), (512, 2.0), (0, 1.0))  # f, g (x2 for 2*sig(2g)-1), i
O_BLOCKS = ((768, 1.0),)
W_FGI = 3 * S  # 192
W_O = S  # 64


def _ensure_paths():
    for p in ("/opt/trn_rl_repo",):
        if os.path.isdir(p) and p not in sys.path:
            sys.path.append(p)


def _pack_tile(w_ih, w_hh, bsum, blocks):
    """Host-side permutation of weights into one gate-group tile layout."""
    ncol = S * len(blocks)
    wih = np.empty((I_DIM, NJ, ncol), np.float32)
    whh = np.empty((128, 2, NJ, ncol), np.float32)
    bias = np.empty((1, NJ, ncol), np.float32)
    # DVE 32x32 block-transpose of packed h puts H-input index
    # 64*(k//32) + 32*u + (k%32) at partition k of lhsT column-group u.
    k = np.arange(128)
    hperm = [64 * (k // 32) + 32 * u + (k % 32) for u in range(2)]
    for q, (rb, scale) in enumerate(blocks):
        for j in range(NJ):
            rows = slice(rb + S * j, rb + S * j + S)
            wih[:, j, S * q : S * q + S] = scale * w_ih[rows, :].T
            for u in range(2):
                whh[:, u, j, S * q : S * q + S] = (
                    scale * w_hh[rows, :][:, hperm[u]].T
                )
            bias[0, j, S * q : S * q + S] = scale * bsum[rows]
    return wih, whh, bias


def _prep_weights(w_ih, w_hh, b_ih, b_hh):
    bsum = (b_ih + b_hh).astype(np.float32)
    fgi = _pack_tile(w_ih, w_hh, bsum, FGI_BLOCKS)
    o = _pack_tile(w_ih, w_hh, bsum, O_BLOCKS)
    ident = np.zeros((128, 32), np.float32)
    for p in range(128):
        ident[p, p % 32] = 1.0
    return fgi, o, ident


def build_nc(T=T_FULL, TC=32, debug=False):
    """Build the per-core Bass program (SPMD: same program on all cores)."""
    _ensure_paths()
    import concourse.bacc as bacc
    import concourse.mybir as mybir
    import concourse.tile as tile
    from contextlib import ExitStack

    fp32 = mybir.dt.float32
    fp16 = mybir.dt.float16
    bf16 = mybir.dt.bfloat16
    AF = mybir.ActivationFunctionType
    ALU = mybir.AluOpType

    assert T % TC == 0 and TC % 8 == 0

    nc = bacc.Bacc("TRN2", target_bir_lowering=False, debug=debug)

    x_d = nc.dram_tensor("x", [B, T, I_DIM], fp32, kind="ExternalInput").ap()
    h0_d = nc.dram_tensor("h0", [B, H], bf16, kind="ExternalInput").ap()
    c0_d = nc.dram_tensor("c0", [B, H], fp32, kind="ExternalInput").ap()
    # x/h weights in bf16: matmuls stream at 1 cycle/row at any N and keep the
    # 4-way PE column-group concurrency (fp32 is 2 half-speed passes; fp32r
    # forbids dst partitions != 0, which the column groups need).
    wih_fgi_d = nc.dram_tensor(
        "wih_fgi", [I_DIM, NJ, W_FGI], bf16, kind="ExternalInput"
    ).ap()
    wih_o_d = nc.dram_tensor(
        "wih_o", [I_DIM, NJ, W_O], bf16, kind="ExternalInput"
    ).ap()
    whh_fgi_d = nc.dram_tensor(
        "whh_fgi", [128, 2, NJ, W_FGI], bf16, kind="ExternalInput"
    ).ap()
    whh_o_d = nc.dram_tensor(
        "whh_o", [128, 2, NJ, W_O], bf16, kind="ExternalInput"
    ).ap()
    # bias split b = b1 + b2 with b1 = bf16(b), b2 = bf16(b - b1): two bf16
    # K=1 rounds reproduce the fp32 bias to ~1e-6 while streaming single-pass.
    b_d = {}
    for nm, w in (("fgi", W_FGI), ("o", W_O)):
        b_d[nm, 1] = nc.dram_tensor(
            f"b1_{nm}", [1, NJ, w], bf16, kind="ExternalInput"
        ).ap()
    ident_d = nc.dram_tensor("ident", [128, 32], fp32, kind="ExternalInput").ap()
    hn_d = nc.dram_tensor("hn", [B, H], fp32, kind="ExternalOutput").ap()

    with tile.TileContext(nc) as tc, ExitStack() as ctx:
        consts = ctx.enter_context(tc.tile_pool(name="consts", bufs=1))
        states = ctx.enter_context(tc.tile_pool(name="states", bufs=1))
        lhsT_pool = ctx.enter_context(tc.tile_pool(name="lhsT", bufs=4))
        x_pool = ctx.enter_context(tc.tile_pool(name="xstream", bufs=2))
        # bufs=5: group g of chunk ch+1 is prepped mid-chunk while group g of
        # chunk ch is still live; 4-back reuse would stall the DVE FIFO on a
        # WAR wait for ~3 steps.
        xT_pool = ctx.enter_context(tc.tile_pool(name="xT", bufs=6))
        ew_pool = ctx.enter_context(tc.tile_pool(name="ew", bufs=4))
        # bufs=3: with 2, the next step's bias round inherits a WAR dep that
        # resolves only at the CURRENT step's last psum read, pushing it (cold)
        # into the critical window.
        g_psum = ctx.enter_context(tc.tile_pool(name="g_psum", bufs=3, space="PSUM"))
        xt_psum = ctx.enter_context(tc.tile_pool(name="xt_psum", bufs=2, space="PSUM"))

        # ---- constants ----
        wih_fgi_sb = consts.tile([I_DIM, NJ, W_FGI], bf16, name="wih_fgi_sb")
        nc.sync.dma_start(out=wih_fgi_sb, in_=wih_fgi_d)
        wih_o_sb = consts.tile([I_DIM, NJ, W_O], bf16, name="wih_o_sb")
        nc.sync.dma_start(out=wih_o_sb, in_=wih_o_d)
        whh_fgi_sb = consts.tile([128, 2, NJ, W_FGI], bf16, name="whh_fgi_sb")
        nc.sync.dma_start(out=whh_fgi_sb, in_=whh_fgi_d)
        whh_o_sb = consts.tile([128, 2, NJ, W_O], bf16, name="whh_o_sb")
        nc.sync.dma_start(out=whh_o_sb, in_=whh_o_d)
        b_sb = {}
        for nm, w in (("fgi", W_FGI), ("o", W_O)):
            t_ = consts.tile([1, NJ, w], bf16, name=f"b1_{nm}_sb")
            nc.sync.dma_start(out=t_, in_=b_d[nm, 1])
            b_sb[nm, 1] = t_
        ident_sb = consts.tile([128, 32], fp32, name="ident_sb")
        nc.sync.dma_start(out=ident_sb, in_=ident_d)
        ones_sb = consts.tile([1, 32], bf16, name="ones_sb")
        nc.vector.memset(ones_sb, 1.0)
        # NOTE: a 17us contiguous burst of back-to-back matmuls measured NO
        # HAM un-throttle on this platform -- the PE streams at 1.2GHz
        # permanently.  No warm-up tricks help; all cost models below assume
        # the cold (219+N)/1.2 latency and N/1.2 pipelined rate.

        # ---- state init (packed) ----
        c_sb = states.tile([128, S], fp32, name="c_sb")
        # h only feeds the gate matmuls (via the transpose), so it lives in
        # bf16; the final step writes a separate fp32 copy for the output.
        h_sb = states.tile([128, S], bf16, name="h_sb")
        for j in range(NJ):
            nc.sync.dma_start(
                out=c_sb[32 * j : 32 * j + 32, :], in_=c0_d[:, S * j : S * j + S]
            )
            nc.sync.dma_start(
                out=h_sb[32 * j : 32 * j + 32, :], in_=h0_d[:, S * j : S * j + S]
            )

        def emit_hT():
            """DVE 32x32 block transpose of packed h -> lhsT column groups."""
            hT = lhsT_pool.tile([128, 2 * 32], bf16, name="hT")
            nc.vector.transpose(out=hT, in_=h_sb)
            return hT

        hT = emit_hT()

        n_chunks = T // TC

        def fetch(ch):
            """Start the async HBM read of one x chunk (prefetched 1 ahead)."""
            x_sb = x_pool.tile([B, TC, I_DIM], fp32, name="x_sb")
            nc.sync.dma_start(out=x_sb, in_=x_d[:, ch * TC : (ch + 1) * TC, :])
            return x_sb

        def prep_pe(x_sb, g8):
            """PE-transpose 8 steps' x into a psum staging tile."""
            xt_ps = xt_psum.tile([128, 8 * 32], fp32, name="xt_ps")
            for v in range(8):
                nc.tensor.transpose(
                    out=xt_ps[:, 32 * v : 32 * v + 32],
                    in_=x_sb[:, g8 * 8 + v, :],
                    identity=ident_sb[0:32, :],
                    tile_position=(0, 0),
                )
            return xt_ps

        def prep_cast(xt_ps):
            # Cast on the SCALAR engine (Copy activation): the DVE version
            # wedged ~420ns into the spine's h/transpose FIFO on prep steps;
            # ACT has ~1.7us of idle between sigma_o and tanh.
            xT_sb = xT_pool.tile([128, 8 * 32], bf16, name="xT_sb")
            nc.scalar.copy(out=xT_sb, in_=xt_ps)
            return xT_sb

        def prep_group(x_sb, g8):
            return prep_cast(prep_pe(x_sb, g8))

        def start_rounds(xT_sl):
            """Open a step's psum accumulation: bias + x rounds (h-independent,
            so they run on the PE as soon as the bank frees, well before hT)."""
            g_fgi = g_psum.tile([128, W_FGI], fp32, name="g_fgi")
            g_o = g_psum.tile([128, W_O], fp32, name="g_o")
            for g_ps, nm, wsb in ((g_fgi, "fgi", wih_fgi_sb), (g_o, "o", wih_o_sb)):
                # Single bf16 bias round: abs err ~2e-4, constant every step,
                # damped by the f<1 recurrence -> ~1e-4 in h.  (The old
                # b1+b2 split cost 8 more K=1 matmuls per step.)
                for j in range(NJ):
                    nc.tensor.matmul(
                        g_ps[32 * j : 32 * j + 32, :],
                        ones_sb, b_sb[nm, 1][0:1, j, :],
                        start=True, stop=False,
                        tile_position=(0, 32 * j), skip_group_check=True,
                    )
                for j in range(NJ):
                    nc.tensor.matmul(
                        g_ps[32 * j : 32 * j + 32, :], xT_sl, wsb[:, j, :],
                        start=False, stop=False,
                        tile_position=(0, 32 * j), skip_group_check=True,
                    )
            return (g_fgi, g_o)

        x_cur = fetch(0)
        xT_tiles = [prep_group(x_cur, g8) for g8 in range(TC // 8)]
        g_ps = None
        for ch in range(n_chunks):
            if ch + 1 < n_chunks:
                x_next = fetch(ch + 1)
            xT_next = [None] * (TC // 8)
            if g_ps is None:
                g_ps = start_rounds(xT_tiles[0][:, 0:32])
            pend_ps = None
            for u in range(TC):
                t = ch * TC + u
                g_fgi, g_o = g_ps
                # Next chunk's x-transposes at the step TOP: in the PE queue
                # they run during the PREVIOUS step's elementwise window
                # (ahead of the h-rounds, which wait on hT anyway).  Emitted
                # after add, they measured ~1us late and their CAST cascaded
                # into the h/transpose tail (+600ns on those steps).
                if u % 8 == 4 and ch + 1 < n_chunks:
                    pend_ps = prep_pe(x_next, (u - 4) // 8)
                # h rounds: the only h_{t-1}-dependent matmuls; fgi tile first
                # (and stopped first) so sigmoid(f,g,i) starts while the o
                # rounds still stream.
                for g_half, whh_sb_ in ((g_fgi, whh_fgi_sb), (g_o, whh_o_sb)):
                    for rnd in range(2):
                        for j in range(NJ):
                            nc.tensor.matmul(
                                g_half[32 * j : 32 * j + 32, :],
                                hT[:, 32 * rnd : 32 * rnd + 32],
                                whh_sb_[:, rnd, j, :],
                                start=False, stop=(rnd == 1),
                                tile_position=(0, 32 * j), skip_group_check=True,
                            )
                # gates: fgi cols [0:64]=f [64:128]=g' [128:192]=i; o tile.
                # fp16 gate tiles: DVE 2-byte fast path without bf16's 8-bit
                # mantissa (the 2sg-1 rewrite would cancel to ~4e-3 abs in
                # bf16; fp16 keeps it ~5e-4).
                sig = ew_pool.tile([128, W_FGI], fp16, name="sig")
                nc.scalar.activation(sig, g_fgi, AF.Sigmoid)
                sig_o = ew_pool.tile([128, S], bf16, name="sig_o")
                nc.scalar.activation(sig_o, g_o, AF.Sigmoid)
                # next-chunk x cast rides ACT's idle window before tanh
                if pend_ps is not None:
                    xT_next[(u - 4) // 8] = prep_cast(pend_ps)
                    pend_ps = None
                # c-chain: c = f*c + (2*sg-1)*i; the (2sg-1)*i product is ONE
                # fused DVE op (affine_mul_reduce: (in0*2-1)*in1).
                uu = ew_pool.tile([128, S], bf16, name="uu")
                uacc = ew_pool.tile([128, 1], fp32, name="uacc")
                nc.vector.affine_mul_reduce(
                    out=uu, accum_out=uacc,
                    in0=sig[:, S : 2 * S], in1=sig[:, 2 * S : 3 * S],
                    scale=2.0, bias=-1.0,
                )
                cf = ew_pool.tile([128, S], fp32, name="cf")
                nc.vector.tensor_mul(cf, sig[:, 0:S], c_sb)
                nc.vector.tensor_add(c_sb, cf, uu)
                tcc = ew_pool.tile([128, S], bf16, name="tcc")
                nc.scalar.activation(tcc, c_sb, AF.Tanh)
                if t < T - 1:
                    # h-mul + 32x32-block transpose in column halves: PE
                    # K-round 0 only needs hT[:, 0:32], so it starts while
                    # the second half is still on the DVE.
                    hT = lhsT_pool.tile([128, 2 * 32], bf16, name="hT")
                    for hh in range(2):
                        cs = slice(32 * hh, 32 * hh + 32)
                        nc.vector.tensor_mul(h_sb[:, cs], sig_o[:, cs], tcc[:, cs])
                        nc.vector.transpose(out=hT[:, cs], in_=h_sb[:, cs])
                else:
                    # full-precision copy of the final h for the output
                    hf_sb = states.tile([128, S], fp32, name="hf_sb")
                    nc.vector.tensor_mul(hf_sb, sig_o, tcc)
                if u < TC - 1:
                    v = u + 1
                    g_ps = start_rounds(
                        xT_tiles[v // 8][:, 32 * (v % 8) : 32 * (v % 8) + 32]
                    )
                else:
                    g_ps = None  # reopened at the next chunk top
            if ch + 1 < n_chunks:
                x_cur = x_next
                xT_tiles = xT_next

        # ---- write back final h (unpack) ----
        for j in range(NJ):
            nc.sync.dma_start(
                out=hn_d[:, S * j : S * j + S], in_=hf_sb[32 * j : 32 * j + 32, :]
            )

    nc.compile()
    return nc


def _shard_inputs(x, h0, c0, w_ih, w_hh, b_ih, b_hh, T=T_FULL):
    import ml_dtypes

    bf16 = ml_dtypes.bfloat16
    (wih_fgi, whh_fgi, bias_fgi), (wih_o, whh_o, bias_o), ident = _prep_weights(
        np.asarray(w_ih, np.float32),
        np.asarray(w_hh, np.float32),
        np.asarray(b_ih, np.float32),
        np.asarray(b_hh, np.float32),
    )

    x = np.asarray(x, np.float32)
    h0 = np.asarray(h0, np.float32)
    c0 = np.asarray(c0, np.float32)
    common = {
        "wih_fgi": wih_fgi.astype(bf16),
        "wih_o": wih_o.astype(bf16),
        "whh_fgi": whh_fgi.astype(bf16),
        "whh_o": whh_o.astype(bf16),
        "b1_fgi": bias_fgi.astype(bf16),
        "b1_o": bias_o.astype(bf16),
        "ident": ident,
    }
    in_maps = []
    for k in range(NCORES):
        bs = slice(B * k, B * (k + 1))
        in_maps.append(
            {
                "x": np.ascontiguousarray(x[bs, :T, :]),
                "h0": np.ascontiguousarray(h0[0, bs, :]).astype(bf16),
                "c0": np.ascontiguousarray(c0[0, bs, :]),
                **common,
            }
        )
    return in_maps


_NC_CACHE = {}


def run_hw(x, h0, c0, w_ih, w_hh, b_ih, b_hh, T=T_FULL, TC=32, trace=False):
    _ensure_paths()
    from concourse.bass_utils import run_bass_kernel_spmd

    key = (T, TC)
    if key not in _NC_CACHE:
        _NC_CACHE[key] = build_nc(T=T, TC=TC)
    nc = _NC_CACHE[key]
    in_maps = _shard_inputs(x, h0, c0, w_ih, w_hh, b_ih, b_hh, T=T)
    res = run_bass_kernel_spmd(nc, in_maps, list(range(NCORES)), trace=trace)
    hn = np.stack([res.results[k]["hn"] for k in range(NCORES)], axis=0)
    return hn.reshape(1, B_TOT, H), res


def kernel(x, h0, c0, w_ih, w_hh, b_ih, b_hh):
    out, _ = run_hw(x, h0, c0, w_ih, w_hh, b_ih, b_hh)
    return out.astype(np.float32)


def _np_reference(x, h0, c0, w_ih, w_hh, b_ih, b_hh, T=None):
    """Numpy oracle for development (matches reference.py)."""
    x = np.asarray(x, np.float64)
    if T is not None:
        x = x[:, :T, :]
    h = np.asarray(h0, np.float64)[0]
    c = np.asarray(c0, np.float64)[0]
    gx = np.einsum("bti,gi->tbg", x, np.asarray(w_ih, np.float64)) + (
        np.asarray(b_ih, np.float64) + np.asarray(b_hh, np.float64)
    )
    W = np.asarray(w_hh, np.float64)

    def sg(v):
        return 1.0 / (1.0 + np.exp(-v))

    for t in range(x.shape[1]):
        g = gx[t] + h @ W.T
        i = sg(g[:, 0:256])
        f = sg(g[:, 256:512])
        gg = np.tanh(g[:, 512:768])
        o = sg(g[:, 768:1024])
        c = f * c + i * gg
        h = o * np.tanh(c)
    return h[None].astype(np.float32)


# revision 35
# speedup vs baseline: 1.1832x; 1.1832x over previous
"""LSTM (single layer, final hidden state) on 8 Trainium2 NeuronCores.

Reference computation (per batch row b):
    gx[t] = x[t] @ w_ih.T + (b_ih + b_hh)
    g     = gx[t] + h @ w_hh.T          # [B, 4H], gate order i,f,g,o
    i,f,o = sigmoid(...), g_c = tanh(...)
    c     = f*c + i*g_c
    h     = o * tanh(c)
returns h after T steps, shape [1, B, H].

Sharding: data-parallel over batch B=256 -> 8 cores x 32. Weights replicated.

Per-core layout ("packed"): partition p = 32*j + b, where j in [0,4) indexes
an H-quarter (H index = 64*j + s, s in [0,64)) and b in [0,32) is the local
batch.  All elementwise tiles are [128, *].

The wall time is T x the per-step dependency-chain latency.  On top of the
previous all-matmul/packing structure, this version crushes the ACT
(scalar-engine) spine cost: every ACT instruction costs ~N*0.83ns + ~150ns
fixed (SBUF/PSUM access pipe), so the 4 activation calls/step of the old
kernel (~1.4us of ACT busy, serialized on the spine) dominate.  Changes:
  * tanh(g) = 2*sigmoid(2g) - 1 with the 2x folded into the g rows of
    w_ih/w_hh/bias host-side -> ALL gates go through ONE Sigmoid call.
  * Gates grouped (f,g,i | o) in TWO psum tiles: sigmoid(f,g,i) [N=192]
    fires as soon as the fgi h-rounds stop (o-rounds still streaming);
    sigmoid(o) [N=64] runs off the c-spine in ACT's idle window.
  * c-chain on DVE: gg=2*sg-1 (tensor_scalar, 4x mode), u=gg*i,
    cf=f*c, c=cf+u; then tanh(c) [N=64] and h=o*tanh(c), hT transpose.
  * Gate tiles in fp16 (not bf16): kills the 2sg-1 cancellation error and
    halves DVE read traffic same as bf16; h stays bf16 for the PE lhsT.
  * c state in fp32 SBUF (DVE access 58cyc vs 120 psum; ACT tanh pays
    +50cyc reading SBUF but the DVE chain saves more).

Everything h-independent stays off the spine as before: bias (two bf16
rounds reproducing the fp32 bias), x rounds for step t+1 opened during step
t's elementwise window, x chunks DMA-prefetched + PE-transposed, and a
zero-contribution warm-up matmul chained mid-window keeps the PE p-state
at 2.4GHz.
"""

import os
import sys

import numpy as np

B_TOT, T_FULL, I_DIM, H = 256, 1024, 128, 256
NCORES = 8
B = B_TOT // NCORES  # 32 per core
NJ = 4  # H quarters
S = H // NJ  # 64
# (row base in the PyTorch i,f,g,o layout, pre-scale) per column block
FGI_BLOCKS = ((256, 1.0), (512, 2.0), (0, 1.0))  # f, g (x2 for 2*sig(2g)-1), i
O_BLOCKS = ((768, 1.0),)
W_FGI = 3 * S  # 192
W_O = S  # 64


def _ensure_paths():
    for p in ("/opt/trn_rl_repo",):
        if os.path.isdir(p) and p not in sys.path:
            sys.path.append(p)


def _pack_tile(w_ih, w_hh, bsum, blocks):
    """Host-side permutation of weights into one gate-group tile layout."""
    ncol = S * len(blocks)
    wih = np.empty((I_DIM, NJ, ncol), np.float32)
    whh = np.empty((128, 2, NJ, ncol), np.float32)
    bias = np.empty((1, NJ, ncol), np.float32)
    # DVE 32x32 block-transpose of packed h puts H-input index
    # 64*(k//32) + 32*u + (k%32) at partition k of lhsT column-group u.
    k = np.arange(128)
    hperm = [64 * (k // 32) + 32 * u + (k % 32) for u in range(2)]
    for q, (rb, scale) in enumerate(blocks):
        for j in range(NJ):
            rows = slice(rb + S * j, rb + S * j + S)
            wih[:, j, S * q : S * q + S] = scale * w_ih[rows, :].T
            for u in range(2):
                whh[:, u, j, S * q : S * q + S] = (
                    scale * w_hh[rows, :][:, hperm[u]].T
                )
            bias[0, j, S * q : S * q + S] = scale * bsum[rows]
    return wih, whh, bias


def _prep_weights(w_ih, w_hh, b_ih, b_hh):
    bsum = (b_ih + b_hh).astype(np.float32)
    fgi = _pack_tile(w_ih, w_hh, bsum, FGI_BLOCKS)
    o = _pack_tile(w_ih, w_hh, bsum, O_BLOCKS)
    ident = np.zeros((128, 32), np.float32)
    for p in range(128):
        ident[p, p % 32] = 1.0
    return fgi, o, ident


def build_nc(T=T_FULL, TC=32, debug=False):
    """Build the per-core Bass program (SPMD: same program on all cores)."""
    _ensure_paths()
    import concourse.bacc as bacc
    import concourse.mybir as mybir
    import concourse.tile as tile
    from contextlib import ExitStack

    fp32 = mybir.dt.float32
    fp16 = mybir.dt.float16
    bf16 = mybir.dt.bfloat16
    AF = mybir.ActivationFunctionType
    ALU = mybir.AluOpType

    assert T % TC == 0 and TC % 8 == 0

    nc = bacc.Bacc("TRN2", target_bir_lowering=False, debug=debug)

    x_d = nc.dram_tensor("x", [B, T, I_DIM], fp32, kind="ExternalInput").ap()
    h0_d = nc.dram_tensor("h0", [B, H], bf16, kind="ExternalInput").ap()
    c0_d = nc.dram_tensor("c0", [B, H], fp32, kind="ExternalInput").ap()
    # x/h weights in bf16: matmuls stream at 1 cycle/row at any N and keep the
    # 4-way PE column-group concurrency (fp32 is 2 half-speed passes; fp32r
    # forbids dst partitions != 0, which the column groups need).
    wih_fgi_d = nc.dram_tensor(
        "wih_fgi", [I_DIM, NJ, W_FGI], bf16, kind="ExternalInput"
    ).ap()
    wih_o_d = nc.dram_tensor(
        "wih_o", [I_DIM, NJ, W_O], bf16, kind="ExternalInput"
    ).ap()
    whh_fgi_d = nc.dram_tensor(
        "whh_fgi", [128, 2, NJ, W_FGI], bf16, kind="ExternalInput"
    ).ap()
    whh_o_d = nc.dram_tensor(
        "whh_o", [128, 2, NJ, W_O], bf16, kind="ExternalInput"
    ).ap()
    # bias split b = b1 + b2 with b1 = bf16(b), b2 = bf16(b - b1): two bf16
    # K=1 rounds reproduce the fp32 bias to ~1e-6 while streaming single-pass.
    b_d = {}
    for nm, w in (("fgi", W_FGI), ("o", W_O)):
        b_d[nm, 1] = nc.dram_tensor(
            f"b1_{nm}", [1, NJ, w], bf16, kind="ExternalInput"
        ).ap()
    ident_d = nc.dram_tensor("ident", [128, 32], fp32, kind="ExternalInput").ap()
    hn_d = nc.dram_tensor("hn", [B, H], fp32, kind="ExternalOutput").ap()

    with tile.TileContext(nc) as tc, ExitStack() as ctx:
        consts = ctx.enter_context(tc.tile_pool(name="consts", bufs=1))
        states = ctx.enter_context(tc.tile_pool(name="states", bufs=1))
        lhsT_pool = ctx.enter_context(tc.tile_pool(name="lhsT", bufs=4))
        x_pool = ctx.enter_context(tc.tile_pool(name="xstream", bufs=2))
        # bufs=5: group g of chunk ch+1 is prepped mid-chunk while group g of
        # chunk ch is still live; 4-back reuse would stall the DVE FIFO on a
        # WAR wait for ~3 steps.
        xT_pool = ctx.enter_context(tc.tile_pool(name="xT", bufs=5))
        ew_pool = ctx.enter_context(tc.tile_pool(name="ew", bufs=4))
        # bufs=3: with 2, the next step's bias round inherits a WAR dep that
        # resolves only at the CURRENT step's last psum read, pushing it (cold)
        # into the critical window.
        g_psum = ctx.enter_context(tc.tile_pool(name="g_psum", bufs=3, space="PSUM"))
        xt_psum = ctx.enter_context(tc.tile_pool(name="xt_psum", bufs=1, space="PSUM"))

        # ---- constants ----
        wih_fgi_sb = consts.tile([I_DIM, NJ, W_FGI], bf16, name="wih_fgi_sb")
        nc.sync.dma_start(out=wih_fgi_sb, in_=wih_fgi_d)
        wih_o_sb = consts.tile([I_DIM, NJ, W_O], bf16, name="wih_o_sb")
        nc.sync.dma_start(out=wih_o_sb, in_=wih_o_d)
        whh_fgi_sb = consts.tile([128, 2, NJ, W_FGI], bf16, name="whh_fgi_sb")
        nc.sync.dma_start(out=whh_fgi_sb, in_=whh_fgi_d)
        whh_o_sb = consts.tile([128, 2, NJ, W_O], bf16, name="whh_o_sb")
        nc.sync.dma_start(out=whh_o_sb, in_=whh_o_d)
        b_sb = {}
        for nm, w in (("fgi", W_FGI), ("o", W_O)):
            t_ = consts.tile([1, NJ, w], bf16, name=f"b1_{nm}_sb")
            nc.sync.dma_start(out=t_, in_=b_d[nm, 1])
            b_sb[nm, 1] = t_
        ident_sb = consts.tile([128, 32], fp32, name="ident_sb")
        nc.sync.dma_start(out=ident_sb, in_=ident_d)
        ones_sb = consts.tile([1, 32], bf16, name="ones_sb")
        nc.vector.memset(ones_sb, 1.0)
        # NOTE: a 17us contiguous burst of back-to-back matmuls measured NO
        # HAM un-throttle on this platform -- the PE streams at 1.2GHz
        # permanently.  No warm-up tricks help; all cost models below assume
        # the cold (219+N)/1.2 latency and N/1.2 pipelined rate.

        # ---- state init (packed) ----
        c_sb = states.tile([128, S], fp32, name="c_sb")
        # h only feeds the gate matmuls (via the transpose), so it lives in
        # bf16; the final step writes a separate fp32 copy for the output.
        h_sb = states.tile([128, S], bf16, name="h_sb")
        for j in range(NJ):
            nc.sync.dma_start(
                out=c_sb[32 * j : 32 * j + 32, :], in_=c0_d[:, S * j : S * j + S]
            )
            nc.sync.dma_start(
                out=h_sb[32 * j : 32 * j + 32, :], in_=h0_d[:, S * j : S * j + S]
            )

        def emit_hT():
            """DVE 32x32 block transpose of packed h -> lhsT column groups."""
            hT = lhsT_pool.tile([128, 2 * 32], bf16, name="hT")
            nc.vector.transpose(out=hT, in_=h_sb)
            return hT

        hT = emit_hT()

        n_chunks = T // TC

        def fetch(ch):
            """Start the async HBM read of one x chunk (prefetched 1 ahead)."""
            x_sb = x_pool.tile([B, TC, I_DIM], fp32, name="x_sb")
            nc.sync.dma_start(out=x_sb, in_=x_d[:, ch * TC : (ch + 1) * TC, :])
            return x_sb

        def prep_pe(x_sb, g8):
            """PE-transpose 8 steps' x into a psum staging tile."""
            xt_ps = xt_psum.tile([128, 8 * 32], fp32, name="xt_ps")
            for v in range(8):
                nc.tensor.transpose(
                    out=xt_ps[:, 32 * v : 32 * v + 32],
                    in_=x_sb[:, g8 * 8 + v, :],
                    identity=ident_sb[0:32, :],
                    tile_position=(0, 0),
                )
            return xt_ps

        def prep_cast(xt_ps):
            xT_sb = xT_pool.tile([128, 8 * 32], bf16, name="xT_sb")
            nc.vector.tensor_copy(out=xT_sb, in_=xt_ps)
            return xT_sb

        def prep_group(x_sb, g8):
            return prep_cast(prep_pe(x_sb, g8))

        def start_rounds(xT_sl):
            """Open a step's psum accumulation: bias + x rounds (h-independent,
            so they run on the PE as soon as the bank frees, well before hT)."""
            g_fgi = g_psum.tile([128, W_FGI], fp32, name="g_fgi")
            g_o = g_psum.tile([128, W_O], fp32, name="g_o")
            for g_ps, nm, wsb in ((g_fgi, "fgi", wih_fgi_sb), (g_o, "o", wih_o_sb)):
                # Single bf16 bias round: abs err ~2e-4, constant every step,
                # damped by the f<1 recurrence -> ~1e-4 in h.  (The old
                # b1+b2 split cost 8 more K=1 matmuls per step.)
                for j in range(NJ):
                    nc.tensor.matmul(
                        g_ps[32 * j : 32 * j + 32, :],
                        ones_sb, b_sb[nm, 1][0:1, j, :],
                        start=True, stop=False,
                        tile_position=(0, 32 * j), skip_group_check=True,
                    )
                for j in range(NJ):
                    nc.tensor.matmul(
                        g_ps[32 * j : 32 * j + 32, :], xT_sl, wsb[:, j, :],
                        start=False, stop=False,
                        tile_position=(0, 32 * j), skip_group_check=True,
                    )
            return (g_fgi, g_o)

        x_cur = fetch(0)
        xT_tiles = [prep_group(x_cur, g8) for g8 in range(TC // 8)]
        g_ps = None
        for ch in range(n_chunks):
            if ch + 1 < n_chunks:
                x_next = fetch(ch + 1)
            xT_next = [None] * (TC // 8)
            if g_ps is None:
                g_ps = start_rounds(xT_tiles[0][:, 0:32])
            pend_ps = None
            for u in range(TC):
                t = ch * TC + u
                g_fgi, g_o = g_ps
                # Next chunk's x-transposes at the step TOP: in the PE queue
                # they run during the PREVIOUS step's elementwise window
                # (ahead of the h-rounds, which wait on hT anyway).  Emitted
                # after add, they measured ~1us late and their CAST cascaded
                # into the h/transpose tail (+600ns on those steps).
                if u % 8 == 4 and ch + 1 < n_chunks:
                    pend_ps = prep_pe(x_next, (u - 4) // 8)
                # h rounds: the only h_{t-1}-dependent matmuls; fgi tile first
                # (and stopped first) so sigmoid(f,g,i) starts while the o
                # rounds still stream.  sigma_fgi is EMITTED before the o
                # rounds so its PE-counter wait threshold cannot get bumped
                # past them (Tile coarsens wait thresholds to the latest
                # emitted instruction on the producer engine).
                def h_rounds(g_half, whh_sb_):
                    for rnd in range(2):
                        for j in range(NJ):
                            nc.tensor.matmul(
                                g_half[32 * j : 32 * j + 32, :],
                                hT[:, 32 * rnd : 32 * rnd + 32],
                                whh_sb_[:, rnd, j, :],
                                start=False, stop=(rnd == 1),
                                tile_position=(0, 32 * j), skip_group_check=True,
                            )

                h_rounds(g_fgi, whh_fgi_sb)
                # gates: fgi cols [0:64]=f [64:128]=g' [128:192]=i; o tile.
                # fp16 gate tiles: DVE 2-byte fast path without bf16's 8-bit
                # mantissa (the 2sg-1 rewrite would cancel to ~4e-3 abs in
                # bf16; fp16 keeps it ~5e-4).
                sig = ew_pool.tile([128, W_FGI], fp16, name="sig")
                nc.scalar.activation(sig, g_fgi, AF.Sigmoid)
                h_rounds(g_o, whh_o_sb)
                sig_o = ew_pool.tile([128, S], bf16, name="sig_o")
                nc.scalar.activation(sig_o, g_o, AF.Sigmoid)
                # c-chain: c = f*c + (2*sg-1)*i; the (2sg-1)*i product is ONE
                # fused DVE op (affine_mul_reduce: (in0*2-1)*in1).
                uu = ew_pool.tile([128, S], bf16, name="uu")
                uacc = ew_pool.tile([128, 1], fp32, name="uacc")
                nc.vector.affine_mul_reduce(
                    out=uu, accum_out=uacc,
                    in0=sig[:, S : 2 * S], in1=sig[:, 2 * S : 3 * S],
                    scale=2.0, bias=-1.0,
                )
                cf = ew_pool.tile([128, S], fp32, name="cf")
                nc.vector.tensor_mul(cf, sig[:, 0:S], c_sb)
                nc.vector.tensor_add(c_sb, cf, uu)
                # The x-transpose CAST for the next chunk rides the DVE idle
                # window while ACT does tanh (CAST ~424ns vs window ~433ns).
                if pend_ps is not None:
                    xT_next[(u - 4) // 8] = prep_cast(pend_ps)
                    pend_ps = None
                tcc = ew_pool.tile([128, S], bf16, name="tcc")
                nc.scalar.activation(tcc, c_sb, AF.Tanh)
                if t < T - 1:
                    # h-mul + 32x32-block transpose in column halves: PE
                    # K-round 0 only needs hT[:, 0:32], so it starts while
                    # the second half is still on the DVE.
                    hT = lhsT_pool.tile([128, 2 * 32], bf16, name="hT")
                    for hh in range(2):
                        cs = slice(32 * hh, 32 * hh + 32)
                        nc.vector.tensor_mul(h_sb[:, cs], sig_o[:, cs], tcc[:, cs])
                        nc.vector.transpose(out=hT[:, cs], in_=h_sb[:, cs])
                else:
                    # full-precision copy of the final h for the output
                    hf_sb = states.tile([128, S], fp32, name="hf_sb")
                    nc.vector.tensor_mul(hf_sb, sig_o, tcc)
                if u < TC - 1:
                    v = u + 1
                    g_ps = start_rounds(
                        xT_tiles[v // 8][:, 32 * (v % 8) : 32 * (v % 8) + 32]
                    )
                else:
                    g_ps = None  # reopened at the next chunk top
            if ch + 1 < n_chunks:
                x_cur = x_next
                xT_tiles = xT_next

        # ---- write back final h (unpack) ----
        for j in range(NJ):
            nc.sync.dma_start(
                out=hn_d[:, S * j : S * j + S], in_=hf_sb[32 * j : 32 * j + 32, :]
            )

    nc.compile()
    return nc


def _shard_inputs(x, h0, c0, w_ih, w_hh, b_ih, b_hh, T=T_FULL):
    import ml_dtypes

    bf16 = ml_dtypes.bfloat16
    (wih_fgi, whh_fgi, bias_fgi), (wih_o, whh_o, bias_o), ident = _prep_weights(
        np.asarray(w_ih, np.float32),
        np.asarray(w_hh, np.float32),
        np.asarray(b_ih, np.float32),
        np.asarray(b_hh, np.float32),
    )

    x = np.asarray(x, np.float32)
    h0 = np.asarray(h0, np.float32)
    c0 = np.asarray(c0, np.float32)
    common = {
        "wih_fgi": wih_fgi.astype(bf16),
        "wih_o": wih_o.astype(bf16),
        "whh_fgi": whh_fgi.astype(bf16),
        "whh_o": whh_o.astype(bf16),
        "b1_fgi": bias_fgi.astype(bf16),
        "b1_o": bias_o.astype(bf16),
        "ident": ident,
    }
    in_maps = []
    for k in range(NCORES):
        bs = slice(B * k, B * (k + 1))
        in_maps.append(
            {
                "x": np.ascontiguousarray(x[bs, :T, :]),
                "h0": np.ascontiguousarray(h0[0, bs, :]).astype(bf16),
                "c0": np.ascontiguousarray(c0[0, bs, :]),
                **common,
            }
        )
    return in_maps


_NC_CACHE = {}


def run_hw(x, h0, c0, w_ih, w_hh, b_ih, b_hh, T=T_FULL, TC=32, trace=False):
    _ensure_paths()
    from concourse.bass_utils import run_bass_kernel_spmd

    key = (T, TC)
    if key not in _NC_CACHE:
        _NC_CACHE[key] = build_nc(T=T, TC=TC)
    nc = _NC_CACHE[key]
    in_maps = _shard_inputs(x, h0, c0, w_ih, w_hh, b_ih, b_hh, T=T)
    res = run_bass_kernel_spmd(nc, in_maps, list(range(NCORES)), trace=trace)
    hn = np.stack([res.results[k]["hn"] for k in range(NCORES)], axis=0)
    return hn.reshape(1, B_TOT, H), res


def kernel(x, h0, c0, w_ih, w_hh, b_ih, b_hh):
    out, _ = run_hw(x, h0, c0, w_ih, w_hh, b_ih, b_hh)
    return out.astype(np.float32)


def _np_reference(x, h0, c0, w_ih, w_hh, b_ih, b_hh, T=None):
    """Numpy oracle for development (matches reference.py)."""
    x = np.asarray(x, np.float64)
    if T is not None:
        x = x[:, :T, :]
    h = np.asarray(h0, np.float64)[0]
    c = np.asarray(c0, np.float64)[0]
    gx = np.einsum("bti,gi->tbg", x, np.asarray(w_ih, np.float64)) + (
        np.asarray(b_ih, np.float64) + np.asarray(b_hh, np.float64)
    )
    W = np.asarray(w_hh, np.float64)

    def sg(v):
        return 1.0 / (1.0 + np.exp(-v))

    for t in range(x.shape[1]):
        g = gx[t] + h @ W.T
        i = sg(g[:, 0:256])
        f = sg(g[:, 256:512])
        gg = np.tanh(g[:, 512:768])
        o = sg(g[:, 768:1024])
        c = f * c + i * gg
        h = o * np.tanh(c)
    return h[None].astype(np.float32)


# revision 40
# speedup vs baseline: 1.1839x; 1.0006x over previous
"""LSTM (single layer, final hidden state) on 8 Trainium2 NeuronCores.

Reference computation (per batch row b):
    gx[t] = x[t] @ w_ih.T + (b_ih + b_hh)
    g     = gx[t] + h @ w_hh.T          # [B, 4H], gate order i,f,g,o
    i,f,o = sigmoid(...), g_c = tanh(...)
    c     = f*c + i*g_c
    h     = o * tanh(c)
returns h after T steps, shape [1, B, H].

Sharding: data-parallel over batch B=256 -> 8 cores x 32. Weights replicated.

Per-core layout ("packed"): partition p = 32*j + b, where j in [0,4) indexes
an H-quarter (H index = 64*j + s, s in [0,64)) and b in [0,32) is the local
batch.  All elementwise tiles are [128, *].

The wall time is T x the per-step dependency-chain latency.  On top of the
previous all-matmul/packing structure, this version crushes the ACT
(scalar-engine) spine cost: every ACT instruction costs ~N*0.83ns + ~150ns
fixed (SBUF/PSUM access pipe), so the 4 activation calls/step of the old
kernel (~1.4us of ACT busy, serialized on the spine) dominate.  Changes:
  * tanh(g) = 2*sigmoid(2g) - 1 with the 2x folded into the g rows of
    w_ih/w_hh/bias host-side -> ALL gates go through ONE Sigmoid call.
  * Gates grouped (f,g,i | o) in TWO psum tiles: sigmoid(f,g,i) [N=192]
    fires as soon as the fgi h-rounds stop (o-rounds still streaming);
    sigmoid(o) [N=64] runs off the c-spine in ACT's idle window.
  * c-chain on DVE: u=(2*sg-1)*si in ONE fused op (affine_mul_reduce),
    cf=f*c, c=cf+u; then tanh(c) [N=64] and h=o*tanh(c), hT transpose.
  * h-mul + 32x32-block transpose in column halves so the PE K-round 0
    starts while the second hT half is still on the DVE.
  * Gate tiles in fp16 (not bf16): kills the 2sg-1 cancellation error and
    halves DVE read traffic same as bf16; h stays bf16 for the PE lhsT.
  * c state in fp32 SBUF (DVE access 58cyc vs 120 psum).
  * Single bf16 bias round (abs err ~2e-4, damped by the f<1 recurrence).
  * sigma_fgi emitted BEFORE the o h-rounds: Tile coarsens sem-wait
    thresholds to the latest emitted producer-engine instruction, so
    emitting it later made sigma wait on the o rounds too.

Everything h-independent stays off the spine: x rounds for step t+1 opened
during step t's elementwise window, x chunks DMA-prefetched, PE-transposed
at the step top of 4 spread steps per chunk, and their psum->SBUF cast
rides the DVE idle window under tanh.

Measured (trace-driven facts for future iterations):
  * The PE never leaves the 4/8 HAM throttle in this environment (a 17us
    contiguous matmul burst stayed at the cold (219+N)/1.2 latency, N/1.2
    pipelined rate) -- model everything at 1.2GHz.
  * ACT instruction: ~(N+310)/1.2 ns; DVE op: ~(N+58)*1.04+45 ns with
    ~150ns issue-to-issue spacing; tensor_tensor_scan runs 2 cyc/elem
    (tried for the c update: net loss, reverted).
  * Engine wait-queues (depth 4) let ready instructions pass a parked one;
    a 400ns cast parked mid-FIFO displaces the h/hT tail on prep steps.

Measured on trn2 via axon: 2961488 ns (vs 3351814 ns for the previous
session's kernel re-measured in this environment), rel err 6.1e-3.
"""

import os
import sys

import numpy as np

B_TOT, T_FULL, I_DIM, H = 256, 1024, 128, 256
NCORES = 8
B = B_TOT // NCORES  # 32 per core
NJ = 4  # H quarters
S = H // NJ  # 64
# (row base in the PyTorch i,f,g,o layout, pre-scale) per column block
FGI_BLOCKS = ((256, 1.0), (512, 2.0), (0, 1.0))  # f, g (x2 for 2*sig(2g)-1), i
O_BLOCKS = ((768, 1.0),)
W_FGI = 3 * S  # 192
W_O = S  # 64


def _ensure_paths():
    for p in ("/opt/trn_rl_repo",):
        if os.path.isdir(p) and p not in sys.path:
            sys.path.append(p)


def _pack_tile(w_ih, w_hh, bsum, blocks):
    """Host-side permutation of weights into one gate-group tile layout."""
    ncol = S * len(blocks)
    wih = np.empty((I_DIM, NJ, ncol), np.float32)
    whh = np.empty((128, 2, NJ, ncol), np.float32)
    bias = np.empty((1, NJ, ncol), np.float32)
    # DVE 32x32 block-transpose of packed h puts H-input index
    # 64*(k//32) + 32*u + (k%32) at partition k of lhsT column-group u.
    k = np.arange(128)
    hperm = [64 * (k // 32) + 32 * u + (k % 32) for u in range(2)]
    for q, (rb, scale) in enumerate(blocks):
        for j in range(NJ):
            rows = slice(rb + S * j, rb + S * j + S)
            wih[:, j, S * q : S * q + S] = scale * w_ih[rows, :].T
            for u in range(2):
                whh[:, u, j, S * q : S * q + S] = (
                    scale * w_hh[rows, :][:, hperm[u]].T
                )
            bias[0, j, S * q : S * q + S] = scale * bsum[rows]
    return wih, whh, bias


def _prep_weights(w_ih, w_hh, b_ih, b_hh):
    bsum = (b_ih + b_hh).astype(np.float32)
    fgi = _pack_tile(w_ih, w_hh, bsum, FGI_BLOCKS)
    o = _pack_tile(w_ih, w_hh, bsum, O_BLOCKS)
    ident = np.zeros((128, 32), np.float32)
    for p in range(128):
        ident[p, p % 32] = 1.0
    return fgi, o, ident


def build_nc(T=T_FULL, TC=32, debug=False):
    """Build the per-core Bass program (SPMD: same program on all cores)."""
    _ensure_paths()
    import concourse.bacc as bacc
    import concourse.mybir as mybir
    import concourse.tile as tile
    from contextlib import ExitStack

    fp32 = mybir.dt.float32
    fp16 = mybir.dt.float16
    bf16 = mybir.dt.bfloat16
    AF = mybir.ActivationFunctionType
    ALU = mybir.AluOpType

    assert T % TC == 0 and TC % 8 == 0

    nc = bacc.Bacc("TRN2", target_bir_lowering=False, debug=debug)

    x_d = nc.dram_tensor("x", [B, T, I_DIM], fp32, kind="ExternalInput").ap()
    h0_d = nc.dram_tensor("h0", [B, H], bf16, kind="ExternalInput").ap()
    c0_d = nc.dram_tensor("c0", [B, H], fp32, kind="ExternalInput").ap()
    # x/h weights in bf16: matmuls stream at 1 cycle/row at any N and keep the
    # 4-way PE column-group concurrency (fp32 is 2 half-speed passes; fp32r
    # forbids dst partitions != 0, which the column groups need).
    wih_fgi_d = nc.dram_tensor(
        "wih_fgi", [I_DIM, NJ, W_FGI], bf16, kind="ExternalInput"
    ).ap()
    wih_o_d = nc.dram_tensor(
        "wih_o", [I_DIM, NJ, W_O], bf16, kind="ExternalInput"
    ).ap()
    whh_fgi_d = nc.dram_tensor(
        "whh_fgi", [128, 2, NJ, W_FGI], bf16, kind="ExternalInput"
    ).ap()
    whh_o_d = nc.dram_tensor(
        "whh_o", [128, 2, NJ, W_O], bf16, kind="ExternalInput"
    ).ap()
    # bias split b = b1 + b2 with b1 = bf16(b), b2 = bf16(b - b1): two bf16
    # K=1 rounds reproduce the fp32 bias to ~1e-6 while streaming single-pass.
    b_d = {}
    for nm, w in (("fgi", W_FGI), ("o", W_O)):
        b_d[nm, 1] = nc.dram_tensor(
            f"b1_{nm}", [1, NJ, w], bf16, kind="ExternalInput"
        ).ap()
    ident_d = nc.dram_tensor("ident", [128, 32], fp32, kind="ExternalInput").ap()
    hn_d = nc.dram_tensor("hn", [B, H], fp32, kind="ExternalOutput").ap()

    with tile.TileContext(nc) as tc, ExitStack() as ctx:
        consts = ctx.enter_context(tc.tile_pool(name="consts", bufs=1))
        states = ctx.enter_context(tc.tile_pool(name="states", bufs=1))
        lhsT_pool = ctx.enter_context(tc.tile_pool(name="lhsT", bufs=4))
        x_pool = ctx.enter_context(tc.tile_pool(name="xstream", bufs=2))
        # bufs=5: group g of chunk ch+1 is prepped mid-chunk while group g of
        # chunk ch is still live; 4-back reuse would stall the DVE FIFO on a
        # WAR wait for ~3 steps.
        xT_pool = ctx.enter_context(tc.tile_pool(name="xT", bufs=5))
        ew_pool = ctx.enter_context(tc.tile_pool(name="ew", bufs=4))
        # bufs=3: with 2, the next step's bias round inherits a WAR dep that
        # resolves only at the CURRENT step's last psum read, pushing it (cold)
        # into the critical window.
        g_psum = ctx.enter_context(tc.tile_pool(name="g_psum", bufs=3, space="PSUM"))
        # bufs=2: with 1, the next prep group's PE transposes inherit a WAR
        # wait on the previous group's cast, which the scheduler then pushes
        # into the spine's h/hT tail on prep steps.
        xt_psum = ctx.enter_context(tc.tile_pool(name="xt_psum", bufs=2, space="PSUM"))

        # ---- constants ----
        wih_fgi_sb = consts.tile([I_DIM, NJ, W_FGI], bf16, name="wih_fgi_sb")
        nc.sync.dma_start(out=wih_fgi_sb, in_=wih_fgi_d)
        wih_o_sb = consts.tile([I_DIM, NJ, W_O], bf16, name="wih_o_sb")
        nc.sync.dma_start(out=wih_o_sb, in_=wih_o_d)
        whh_fgi_sb = consts.tile([128, 2, NJ, W_FGI], bf16, name="whh_fgi_sb")
        nc.sync.dma_start(out=whh_fgi_sb, in_=whh_fgi_d)
        whh_o_sb = consts.tile([128, 2, NJ, W_O], bf16, name="whh_o_sb")
        nc.sync.dma_start(out=whh_o_sb, in_=whh_o_d)
        b_sb = {}
        for nm, w in (("fgi", W_FGI), ("o", W_O)):
            t_ = consts.tile([1, NJ, w], bf16, name=f"b1_{nm}_sb")
            nc.sync.dma_start(out=t_, in_=b_d[nm, 1])
            b_sb[nm, 1] = t_
        ident_sb = consts.tile([128, 32], fp32, name="ident_sb")
        nc.sync.dma_start(out=ident_sb, in_=ident_d)
        ones_sb = consts.tile([1, 32], bf16, name="ones_sb")
        nc.vector.memset(ones_sb, 1.0)
        # NOTE: a 17us contiguous burst of back-to-back matmuls measured NO
        # HAM un-throttle on this platform -- the PE streams at 1.2GHz
        # permanently.  No warm-up tricks help; all cost models below assume
        # the cold (219+N)/1.2 latency and N/1.2 pipelined rate.

        # ---- state init (packed) ----
        c_sb = states.tile([128, S], fp32, name="c_sb")
        # h only feeds the gate matmuls (via the transpose), so it lives in
        # bf16; the final step writes a separate fp32 copy for the output.
        h_sb = states.tile([128, S], bf16, name="h_sb")
        for j in range(NJ):
            nc.sync.dma_start(
                out=c_sb[32 * j : 32 * j + 32, :], in_=c0_d[:, S * j : S * j + S]
            )
            nc.sync.dma_start(
                out=h_sb[32 * j : 32 * j + 32, :], in_=h0_d[:, S * j : S * j + S]
            )

        def emit_hT():
            """DVE 32x32 block transpose of packed h -> lhsT column groups."""
            hT = lhsT_pool.tile([128, 2 * 32], bf16, name="hT")
            nc.vector.transpose(out=hT, in_=h_sb)
            return hT

        hT = emit_hT()

        n_chunks = T // TC

        def fetch(ch):
            """Start the async HBM read of one x chunk (prefetched 1 ahead)."""
            x_sb = x_pool.tile([B, TC, I_DIM], fp32, name="x_sb")
            nc.sync.dma_start(out=x_sb, in_=x_d[:, ch * TC : (ch + 1) * TC, :])
            return x_sb

        def prep_pe(x_sb, g8):
            """PE-transpose 8 steps' x into a psum staging tile."""
            xt_ps = xt_psum.tile([128, 8 * 32], fp32, name="xt_ps")
            for v in range(8):
                nc.tensor.transpose(
                    out=xt_ps[:, 32 * v : 32 * v + 32],
                    in_=x_sb[:, g8 * 8 + v, :],
                    identity=ident_sb[0:32, :],
                    tile_position=(0, 0),
                )
            return xt_ps

        def prep_cast(xt_ps):
            xT_sb = xT_pool.tile([128, 8 * 32], bf16, name="xT_sb")
            nc.vector.tensor_copy(out=xT_sb, in_=xt_ps)
            return xT_sb

        def prep_group(x_sb, g8):
            return prep_cast(prep_pe(x_sb, g8))

        def start_rounds(xT_sl):
            """Open a step's psum accumulation: bias + x rounds (h-independent,
            so they run on the PE as soon as the bank frees, well before hT)."""
            g_fgi = g_psum.tile([128, W_FGI], fp32, name="g_fgi")
            g_o = g_psum.tile([128, W_O], fp32, name="g_o")
            for g_ps, nm, wsb in ((g_fgi, "fgi", wih_fgi_sb), (g_o, "o", wih_o_sb)):
                # Single bf16 bias round: abs err ~2e-4, constant every step,
                # damped by the f<1 recurrence -> ~1e-4 in h.  (The old
                # b1+b2 split cost 8 more K=1 matmuls per step.)
                for j in range(NJ):
                    nc.tensor.matmul(
                        g_ps[32 * j : 32 * j + 32, :],
                        ones_sb, b_sb[nm, 1][0:1, j, :],
                        start=True, stop=False,
                        tile_position=(0, 32 * j), skip_group_check=True,
                    )
                for j in range(NJ):
                    nc.tensor.matmul(
                        g_ps[32 * j : 32 * j + 32, :], xT_sl, wsb[:, j, :],
                        start=False, stop=False,
                        tile_position=(0, 32 * j), skip_group_check=True,
                    )
            return (g_fgi, g_o)

        x_cur = fetch(0)
        xT_tiles = [prep_group(x_cur, g8) for g8 in range(TC // 8)]
        g_ps = None
        for ch in range(n_chunks):
            if ch + 1 < n_chunks:
                x_next = fetch(ch + 1)
            xT_next = [None] * (TC // 8)
            if g_ps is None:
                g_ps = start_rounds(xT_tiles[0][:, 0:32])
            for u in range(TC):
                t = ch * TC + u
                g_fgi, g_o = g_ps
                # Next chunk's x-transposes at the step TOP: in the PE queue
                # they run during the PREVIOUS step's elementwise window
                # (ahead of the h-rounds, which wait on hT anyway).  Emitted
                # after add, they measured ~1us late and their CAST cascaded
                # into the h/transpose tail (+600ns on those steps).
                if u % 8 == 4 and ch + 1 < n_chunks:
                    # cast emitted right behind its producer transposes: its
                    # PE-counter wait then covers ONLY them (not this step's
                    # h/o rounds, where Tile's coarsened threshold previously
                    # landed), so it runs in the DVE idle gap before the
                    # c-chain instead of displacing the h/hT tail.
                    xT_next[(u - 4) // 8] = prep_cast(
                        prep_pe(x_next, (u - 4) // 8)
                    )
                # h rounds: the only h_{t-1}-dependent matmuls; fgi tile first
                # (and stopped first) so sigmoid(f,g,i) starts while the o
                # rounds still stream.  sigma_fgi is EMITTED before the o
                # rounds so its PE-counter wait threshold cannot get bumped
                # past them (Tile coarsens wait thresholds to the latest
                # emitted instruction on the producer engine).
                def h_rounds(g_half, whh_sb_):
                    for rnd in range(2):
                        for j in range(NJ):
                            nc.tensor.matmul(
                                g_half[32 * j : 32 * j + 32, :],
                                hT[:, 32 * rnd : 32 * rnd + 32],
                                whh_sb_[:, rnd, j, :],
                                start=False, stop=(rnd == 1),
                                tile_position=(0, 32 * j), skip_group_check=True,
                            )

                h_rounds(g_fgi, whh_fgi_sb)
                # gates: fgi cols [0:64]=f [64:128]=g' [128:192]=i; o tile.
                # fp16 gate tiles: DVE 2-byte fast path without bf16's 8-bit
                # mantissa (the 2sg-1 rewrite would cancel to ~4e-3 abs in
                # bf16; fp16 keeps it ~5e-4).
                sig = ew_pool.tile([128, W_FGI], fp16, name="sig")
                nc.scalar.activation(sig, g_fgi, AF.Sigmoid)
                h_rounds(g_o, whh_o_sb)
                sig_o = ew_pool.tile([128, S], bf16, name="sig_o")
                nc.scalar.activation(sig_o, g_o, AF.Sigmoid)
                # c-chain: c = f*c + (2*sg-1)*i; the (2sg-1)*i product is ONE
                # fused DVE op (affine_mul_reduce: (in0*2-1)*in1).
                uu = ew_pool.tile([128, S], bf16, name="uu")
                uacc = ew_pool.tile([128, 1], fp32, name="uacc")
                nc.vector.affine_mul_reduce(
                    out=uu, accum_out=uacc,
                    in0=sig[:, S : 2 * S], in1=sig[:, 2 * S : 3 * S],
                    scale=2.0, bias=-1.0,
                )
                cf = ew_pool.tile([128, S], fp32, name="cf")
                nc.vector.tensor_mul(cf, sig[:, 0:S], c_sb)
                nc.vector.tensor_add(c_sb, cf, uu)
                tcc = ew_pool.tile([128, S], bf16, name="tcc")
                nc.scalar.activation(tcc, c_sb, AF.Tanh)
                if t < T - 1:
                    # h-mul + 32x32-block transpose in column halves: PE
                    # K-round 0 only needs hT[:, 0:32], so it starts while
                    # the second half is still on the DVE.
                    hT = lhsT_pool.tile([128, 2 * 32], bf16, name="hT")
                    for hh in range(2):
                        cs = slice(32 * hh, 32 * hh + 32)
                        nc.vector.tensor_mul(h_sb[:, cs], sig_o[:, cs], tcc[:, cs])
                        nc.vector.transpose(out=hT[:, cs], in_=h_sb[:, cs])
                else:
                    # full-precision copy of the final h for the output
                    hf_sb = states.tile([128, S], fp32, name="hf_sb")
                    nc.vector.tensor_mul(hf_sb, sig_o, tcc)
                if u < TC - 1:
                    v = u + 1
                    g_ps = start_rounds(
                        xT_tiles[v // 8][:, 32 * (v % 8) : 32 * (v % 8) + 32]
                    )
                else:
                    g_ps = None  # reopened at the next chunk top
            if ch + 1 < n_chunks:
                x_cur = x_next
                xT_tiles = xT_next

        # ---- write back final h (unpack) ----
        for j in range(NJ):
            nc.sync.dma_start(
                out=hn_d[:, S * j : S * j + S], in_=hf_sb[32 * j : 32 * j + 32, :]
            )

    nc.compile()
    return nc


def _shard_inputs(x, h0, c0, w_ih, w_hh, b_ih, b_hh, T=T_FULL):
    import ml_dtypes

    bf16 = ml_dtypes.bfloat16
    (wih_fgi, whh_fgi, bias_fgi), (wih_o, whh_o, bias_o), ident = _prep_weights(
        np.asarray(w_ih, np.float32),
        np.asarray(w_hh, np.float32),
        np.asarray(b_ih, np.float32),
        np.asarray(b_hh, np.float32),
    )

    x = np.asarray(x, np.float32)
    h0 = np.asarray(h0, np.float32)
    c0 = np.asarray(c0, np.float32)
    common = {
        "wih_fgi": wih_fgi.astype(bf16),
        "wih_o": wih_o.astype(bf16),
        "whh_fgi": whh_fgi.astype(bf16),
        "whh_o": whh_o.astype(bf16),
        "b1_fgi": bias_fgi.astype(bf16),
        "b1_o": bias_o.astype(bf16),
        "ident": ident,
    }
    in_maps = []
    for k in range(NCORES):
        bs = slice(B * k, B * (k + 1))
        in_maps.append(
            {
                "x": np.ascontiguousarray(x[bs, :T, :]),
                "h0": np.ascontiguousarray(h0[0, bs, :]).astype(bf16),
                "c0": np.ascontiguousarray(c0[0, bs, :]),
                **common,
            }
        )
    return in_maps


_NC_CACHE = {}


def run_hw(x, h0, c0, w_ih, w_hh, b_ih, b_hh, T=T_FULL, TC=32, trace=False):
    _ensure_paths()
    from concourse.bass_utils import run_bass_kernel_spmd

    key = (T, TC)
    if key not in _NC_CACHE:
        _NC_CACHE[key] = build_nc(T=T, TC=TC)
    nc = _NC_CACHE[key]
    in_maps = _shard_inputs(x, h0, c0, w_ih, w_hh, b_ih, b_hh, T=T)
    res = run_bass_kernel_spmd(nc, in_maps, list(range(NCORES)), trace=trace)
    hn = np.stack([res.results[k]["hn"] for k in range(NCORES)], axis=0)
    return hn.reshape(1, B_TOT, H), res


def kernel(x, h0, c0, w_ih, w_hh, b_ih, b_hh):
    out, _ = run_hw(x, h0, c0, w_ih, w_hh, b_ih, b_hh)
    return out.astype(np.float32)


def _np_reference(x, h0, c0, w_ih, w_hh, b_ih, b_hh, T=None):
    """Numpy oracle for development (matches reference.py)."""
    x = np.asarray(x, np.float64)
    if T is not None:
        x = x[:, :T, :]
    h = np.asarray(h0, np.float64)[0]
    c = np.asarray(c0, np.float64)[0]
    gx = np.einsum("bti,gi->tbg", x, np.asarray(w_ih, np.float64)) + (
        np.asarray(b_ih, np.float64) + np.asarray(b_hh, np.float64)
    )
    W = np.asarray(w_hh, np.float64)

    def sg(v):
        return 1.0 / (1.0 + np.exp(-v))

    for t in range(x.shape[1]):
        g = gx[t] + h @ W.T
        i = sg(g[:, 0:256])
        f = sg(g[:, 256:512])
        gg = np.tanh(g[:, 512:768])
        o = sg(g[:, 768:1024])
        c = f * c + i * gg
        h = o * np.tanh(c)
    return h[None].astype(np.float32)


# revision 44
# speedup vs baseline: 1.1929x; 1.0076x over previous
"""LSTM (single layer, final hidden state) on 8 Trainium2 NeuronCores.

Reference computation (per batch row b):
    gx[t] = x[t] @ w_ih.T + (b_ih + b_hh)
    g     = gx[t] + h @ w_hh.T          # [B, 4H], gate order i,f,g,o
    i,f,o = sigmoid(...), g_c = tanh(...)
    c     = f*c + i*g_c
    h     = o * tanh(c)
returns h after T steps, shape [1, B, H].

Sharding: data-parallel over batch B=256 -> 8 cores x 32. Weights replicated.

Per-core layout ("packed"): partition p = 32*j + b, where j in [0,4) indexes
an H-quarter (H index = 64*j + s, s in [0,64)) and b in [0,32) is the local
batch.  All elementwise tiles are [128, *].

The wall time is T x the per-step dependency-chain latency.  On top of the
previous all-matmul/packing structure, this version crushes the ACT
(scalar-engine) spine cost: every ACT instruction costs ~N*0.83ns + ~150ns
fixed (SBUF/PSUM access pipe), so the 4 activation calls/step of the old
kernel (~1.4us of ACT busy, serialized on the spine) dominate.  Changes:
  * tanh(g) = 2*sigmoid(2g) - 1 with the 2x folded into the g rows of
    w_ih/w_hh/bias host-side -> ALL gates go through ONE Sigmoid call.
  * Gates grouped (f,g,i | o) in TWO psum tiles: sigmoid(f,g,i) [N=192]
    fires as soon as the fgi h-rounds stop (o-rounds still streaming);
    sigmoid(o) [N=64] runs off the c-spine in ACT's idle window.
  * c-chain on DVE: u=(2*sg-1)*si in ONE fused op (affine_mul_reduce),
    cf=f*c, c=cf+u; then tanh(c) [N=64] and h=o*tanh(c), hT transpose.
  * h-mul + 32x32-block transpose in column halves so the PE K-round 0
    starts while the second hT half is still on the DVE.
  * Gate tiles in fp16 (not bf16): kills the 2sg-1 cancellation error and
    halves DVE read traffic same as bf16; h stays bf16 for the PE lhsT.
  * c state in fp32 SBUF (DVE access 58cyc vs 120 psum).
  * Single bf16 bias round (abs err ~2e-4, damped by the f<1 recurrence).
  * sigma_fgi emitted BEFORE the o h-rounds: Tile coarsens sem-wait
    thresholds to the latest emitted producer-engine instruction, so
    emitting it later made sigma wait on the o rounds too.

Everything h-independent stays off the spine: x rounds for step t+1 opened
during step t's elementwise window, x chunks DMA-prefetched, PE-transposed
at the step top of 4 spread steps per chunk, and their psum->SBUF cast
rides the DVE idle window under tanh.

Measured (trace-driven facts for future iterations):
  * The PE never leaves the 4/8 HAM throttle in this environment (a 17us
    contiguous matmul burst stayed at the cold (219+N)/1.2 latency, N/1.2
    pipelined rate) -- model everything at 1.2GHz.
  * ACT instruction: ~(N+310)/1.2 ns; DVE op: ~(N+58)*1.04+45 ns with
    ~150ns issue-to-issue spacing; tensor_tensor_scan runs 2 cyc/elem
    (tried for the c update: net loss, reverted).
  * Engine wait-queues (depth 4) let ready instructions pass a parked one;
    a 400ns cast parked mid-FIFO displaces the h/hT tail on prep steps.

Measured on trn2 via axon: 2961488 ns (vs 3351814 ns for the previous
session's kernel re-measured in this environment), rel err 6.1e-3.
"""

import os
import sys

import numpy as np

B_TOT, T_FULL, I_DIM, H = 256, 1024, 128, 256
NCORES = 8
B = B_TOT // NCORES  # 32 per core
NJ = 4  # H quarters
S = H // NJ  # 64
# (row base in the PyTorch i,f,g,o layout, pre-scale) per column block
FGI_BLOCKS = ((256, 1.0), (512, 2.0), (0, 1.0))  # f, g (x2 for 2*sig(2g)-1), i
O_BLOCKS = ((768, 1.0),)
W_FGI = 3 * S  # 192
W_O = S  # 64


def _ensure_paths():
    for p in ("/opt/trn_rl_repo",):
        if os.path.isdir(p) and p not in sys.path:
            sys.path.append(p)


def _pack_tile(w_ih, w_hh, bsum, blocks):
    """Host-side permutation of weights into one gate-group tile layout."""
    ncol = S * len(blocks)
    wih = np.empty((I_DIM, NJ, ncol), np.float32)
    whh = np.empty((128, 2, NJ, ncol), np.float32)
    bias = np.empty((1, NJ, ncol), np.float32)
    # DVE 32x32 block-transpose of packed h puts H-input index
    # 64*(k//32) + 32*u + (k%32) at partition k of lhsT column-group u.
    k = np.arange(128)
    hperm = [64 * (k // 32) + 32 * u + (k % 32) for u in range(2)]
    for q, (rb, scale) in enumerate(blocks):
        for j in range(NJ):
            rows = slice(rb + S * j, rb + S * j + S)
            wih[:, j, S * q : S * q + S] = scale * w_ih[rows, :].T
            for u in range(2):
                whh[:, u, j, S * q : S * q + S] = (
                    scale * w_hh[rows, :][:, hperm[u]].T
                )
            bias[0, j, S * q : S * q + S] = scale * bsum[rows]
    return wih, whh, bias


def _prep_weights(w_ih, w_hh, b_ih, b_hh):
    bsum = (b_ih + b_hh).astype(np.float32)
    fgi = _pack_tile(w_ih, w_hh, bsum, FGI_BLOCKS)
    o = _pack_tile(w_ih, w_hh, bsum, O_BLOCKS)
    ident = np.zeros((128, 32), np.float32)
    for p in range(128):
        ident[p, p % 32] = 1.0
    return fgi, o, ident


def build_nc(T=T_FULL, TC=32, debug=False):
    """Build the per-core Bass program (SPMD: same program on all cores)."""
    _ensure_paths()
    import concourse.bacc as bacc
    import concourse.mybir as mybir
    import concourse.tile as tile
    from contextlib import ExitStack

    fp32 = mybir.dt.float32
    fp16 = mybir.dt.float16
    bf16 = mybir.dt.bfloat16
    AF = mybir.ActivationFunctionType
    ALU = mybir.AluOpType

    assert T % TC == 0 and TC % 8 == 0

    nc = bacc.Bacc("TRN2", target_bir_lowering=False, debug=debug)

    x_d = nc.dram_tensor("x", [B, T, I_DIM], fp32, kind="ExternalInput").ap()
    h0_d = nc.dram_tensor("h0", [B, H], bf16, kind="ExternalInput").ap()
    c0_d = nc.dram_tensor("c0", [B, H], fp32, kind="ExternalInput").ap()
    # x/h weights in bf16: matmuls stream at 1 cycle/row at any N and keep the
    # 4-way PE column-group concurrency (fp32 is 2 half-speed passes; fp32r
    # forbids dst partitions != 0, which the column groups need).
    wih_fgi_d = nc.dram_tensor(
        "wih_fgi", [I_DIM, NJ, W_FGI], bf16, kind="ExternalInput"
    ).ap()
    wih_o_d = nc.dram_tensor(
        "wih_o", [I_DIM, NJ, W_O], bf16, kind="ExternalInput"
    ).ap()
    whh_fgi_d = nc.dram_tensor(
        "whh_fgi", [128, 2, NJ, W_FGI], bf16, kind="ExternalInput"
    ).ap()
    whh_o_d = nc.dram_tensor(
        "whh_o", [128, 2, NJ, W_O], bf16, kind="ExternalInput"
    ).ap()
    # bias split b = b1 + b2 with b1 = bf16(b), b2 = bf16(b - b1): two bf16
    # K=1 rounds reproduce the fp32 bias to ~1e-6 while streaming single-pass.
    b_d = {}
    for nm, w in (("fgi", W_FGI), ("o", W_O)):
        b_d[nm, 1] = nc.dram_tensor(
            f"b1_{nm}", [1, NJ, w], bf16, kind="ExternalInput"
        ).ap()
    ident_d = nc.dram_tensor("ident", [128, 32], fp32, kind="ExternalInput").ap()
    hn_d = nc.dram_tensor("hn", [B, H], fp32, kind="ExternalOutput").ap()

    with tile.TileContext(nc) as tc, ExitStack() as ctx:
        consts = ctx.enter_context(tc.tile_pool(name="consts", bufs=1))
        states = ctx.enter_context(tc.tile_pool(name="states", bufs=1))
        lhsT_pool = ctx.enter_context(tc.tile_pool(name="lhsT", bufs=6))
        x_pool = ctx.enter_context(tc.tile_pool(name="xstream", bufs=2))
        # bufs=5: group g of chunk ch+1 is prepped mid-chunk while group g of
        # chunk ch is still live; 4-back reuse would stall the DVE FIFO on a
        # WAR wait for ~3 steps.
        xT_pool = ctx.enter_context(tc.tile_pool(name="xT", bufs=5))
        ew_pool = ctx.enter_context(tc.tile_pool(name="ew", bufs=6))
        # bufs=3: with 2, the next step's bias round inherits a WAR dep that
        # resolves only at the CURRENT step's last psum read, pushing it (cold)
        # into the critical window.
        # bufs=3 is the PSUM ceiling: tiles are bank-granular (fgi+o = 2
        # banks/step), 3 gens + 2 xt staging banks = all 8 banks.
        g_psum = ctx.enter_context(tc.tile_pool(name="g_psum", bufs=3, space="PSUM"))
        # bufs=2: with 1, the next prep group's PE transposes inherit a WAR
        # wait on the previous group's cast, which the scheduler then pushes
        # into the spine's h/hT tail on prep steps.
        xt_psum = ctx.enter_context(tc.tile_pool(name="xt_psum", bufs=2, space="PSUM"))

        # ---- constants ----
        wih_fgi_sb = consts.tile([I_DIM, NJ, W_FGI], bf16, name="wih_fgi_sb")
        nc.sync.dma_start(out=wih_fgi_sb, in_=wih_fgi_d)
        wih_o_sb = consts.tile([I_DIM, NJ, W_O], bf16, name="wih_o_sb")
        nc.sync.dma_start(out=wih_o_sb, in_=wih_o_d)
        whh_fgi_sb = consts.tile([128, 2, NJ, W_FGI], bf16, name="whh_fgi_sb")
        nc.sync.dma_start(out=whh_fgi_sb, in_=whh_fgi_d)
        whh_o_sb = consts.tile([128, 2, NJ, W_O], bf16, name="whh_o_sb")
        nc.sync.dma_start(out=whh_o_sb, in_=whh_o_d)
        b_sb = {}
        for nm, w in (("fgi", W_FGI), ("o", W_O)):
            t_ = consts.tile([1, NJ, w], bf16, name=f"b1_{nm}_sb")
            nc.sync.dma_start(out=t_, in_=b_d[nm, 1])
            b_sb[nm, 1] = t_
        ident_sb = consts.tile([128, 32], fp32, name="ident_sb")
        nc.sync.dma_start(out=ident_sb, in_=ident_d)
        ones_sb = consts.tile([1, 32], bf16, name="ones_sb")
        nc.vector.memset(ones_sb, 1.0)
        # NOTE: a 17us contiguous burst of back-to-back matmuls measured NO
        # HAM un-throttle on this platform -- the PE streams at 1.2GHz
        # permanently.  No warm-up tricks help; all cost models below assume
        # the cold (219+N)/1.2 latency and N/1.2 pipelined rate.

        # ---- state init (packed) ----
        c_sb = states.tile([128, S], fp32, name="c_sb")
        # h only feeds the gate matmuls (via the transpose), so it lives in
        # bf16; the final step writes a separate fp32 copy for the output.
        h_sb = states.tile([128, S], bf16, name="h_sb")
        for j in range(NJ):
            nc.sync.dma_start(
                out=c_sb[32 * j : 32 * j + 32, :], in_=c0_d[:, S * j : S * j + S]
            )
            nc.sync.dma_start(
                out=h_sb[32 * j : 32 * j + 32, :], in_=h0_d[:, S * j : S * j + S]
            )

        def emit_hT():
            """DVE 32x32 block transpose of packed h -> lhsT column groups."""
            hT = lhsT_pool.tile([128, 2 * 32], bf16, name="hT")
            nc.vector.transpose(out=hT, in_=h_sb)
            return hT

        hT = emit_hT()

        n_chunks = T // TC

        def fetch(ch):
            """Start the async HBM read of one x chunk (prefetched 1 ahead)."""
            x_sb = x_pool.tile([B, TC, I_DIM], fp32, name="x_sb")
            nc.sync.dma_start(out=x_sb, in_=x_d[:, ch * TC : (ch + 1) * TC, :])
            return x_sb

        def prep_pe(x_sb, g8):
            """PE-transpose 8 steps' x into a psum staging tile."""
            xt_ps = xt_psum.tile([128, 8 * 32], fp32, name="xt_ps")
            for v in range(8):
                nc.tensor.transpose(
                    out=xt_ps[:, 32 * v : 32 * v + 32],
                    in_=x_sb[:, g8 * 8 + v, :],
                    identity=ident_sb[0:32, :],
                    tile_position=(0, 0),
                )
            return xt_ps

        def prep_cast(xt_ps):
            xT_sb = xT_pool.tile([128, 8 * 32], bf16, name="xT_sb")
            nc.vector.tensor_copy(out=xT_sb, in_=xt_ps)
            return xT_sb

        def prep_group(x_sb, g8):
            return prep_cast(prep_pe(x_sb, g8))

        def start_rounds(xT_sl):
            """Open a step's psum accumulation: bias + x rounds (h-independent,
            so they run on the PE as soon as the bank frees, well before hT)."""
            g_fgi = g_psum.tile([128, W_FGI], fp32, name="g_fgi")
            g_o = g_psum.tile([128, W_O], fp32, name="g_o")
            for g_ps, nm, wsb in ((g_fgi, "fgi", wih_fgi_sb), (g_o, "o", wih_o_sb)):
                # Single bf16 bias round: abs err ~2e-4, constant every step,
                # damped by the f<1 recurrence -> ~1e-4 in h.  (The old
                # b1+b2 split cost 8 more K=1 matmuls per step.)
                for j in range(NJ):
                    nc.tensor.matmul(
                        g_ps[32 * j : 32 * j + 32, :],
                        ones_sb, b_sb[nm, 1][0:1, j, :],
                        start=True, stop=False,
                        tile_position=(0, 32 * j), skip_group_check=True,
                    )
                for j in range(NJ):
                    nc.tensor.matmul(
                        g_ps[32 * j : 32 * j + 32, :], xT_sl, wsb[:, j, :],
                        start=False, stop=False,
                        tile_position=(0, 32 * j), skip_group_check=True,
                    )
            return (g_fgi, g_o)

        x_cur = fetch(0)
        xT_tiles = [prep_group(x_cur, g8) for g8 in range(TC // 8)]
        g_ps = None
        for ch in range(n_chunks):
            if ch + 1 < n_chunks:
                x_next = fetch(ch + 1)
            xT_next = [None] * (TC // 8)
            if g_ps is None:
                g_ps = start_rounds(xT_tiles[0][:, 0:32])
            for u in range(TC):
                t = ch * TC + u
                g_fgi, g_o = g_ps
                # Next chunk's x-transposes at the step TOP: in the PE queue
                # they run during the PREVIOUS step's elementwise window
                # (ahead of the h-rounds, which wait on hT anyway).  Emitted
                # after add, they measured ~1us late and their CAST cascaded
                # into the h/transpose tail (+600ns on those steps).
                if u % 8 == 4 and ch + 1 < n_chunks:
                    # cast emitted right behind its producer transposes: its
                    # PE-counter wait then covers ONLY them (not this step's
                    # h/o rounds, where Tile's coarsened threshold previously
                    # landed), so it runs in the DVE idle gap before the
                    # c-chain instead of displacing the h/hT tail.
                    xT_next[(u - 4) // 8] = prep_cast(
                        prep_pe(x_next, (u - 4) // 8)
                    )
                # h rounds: the only h_{t-1}-dependent matmuls; fgi tile first
                # (and stopped first) so sigmoid(f,g,i) starts while the o
                # rounds still stream.  sigma_fgi is EMITTED before the o
                # rounds so its PE-counter wait threshold cannot get bumped
                # past them (Tile coarsens wait thresholds to the latest
                # emitted instruction on the producer engine).
                def h_rounds(g_half, whh_sb_):
                    for rnd in range(2):
                        for j in range(NJ):
                            nc.tensor.matmul(
                                g_half[32 * j : 32 * j + 32, :],
                                hT[:, 32 * rnd : 32 * rnd + 32],
                                whh_sb_[:, rnd, j, :],
                                start=False, stop=(rnd == 1),
                                tile_position=(0, 32 * j), skip_group_check=True,
                            )

                h_rounds(g_fgi, whh_fgi_sb)
                # gates: fgi cols [0:64]=f [64:128]=g' [128:192]=i; o tile.
                # fp16 gate tiles: DVE 2-byte fast path without bf16's 8-bit
                # mantissa (the 2sg-1 rewrite would cancel to ~4e-3 abs in
                # bf16; fp16 keeps it ~5e-4).
                sig = ew_pool.tile([128, W_FGI], fp16, name="sig")
                nc.scalar.activation(sig, g_fgi, AF.Sigmoid)
                h_rounds(g_o, whh_o_sb)
                sig_o = ew_pool.tile([128, S], bf16, name="sig_o")
                nc.scalar.activation(sig_o, g_o, AF.Sigmoid)
                # c-chain: c = f*c + (2*sg-1)*i; the (2sg-1)*i product is ONE
                # fused DVE op (affine_mul_reduce: (in0*2-1)*in1).
                uu = ew_pool.tile([128, S], bf16, name="uu")
                uacc = ew_pool.tile([128, 1], fp32, name="uacc")
                nc.vector.affine_mul_reduce(
                    out=uu, accum_out=uacc,
                    in0=sig[:, S : 2 * S], in1=sig[:, 2 * S : 3 * S],
                    scale=2.0, bias=-1.0,
                )
                cf = ew_pool.tile([128, S], fp32, name="cf")
                nc.vector.tensor_mul(cf, sig[:, 0:S], c_sb)
                nc.vector.tensor_add(c_sb, cf, uu)
                tcc = ew_pool.tile([128, S], bf16, name="tcc")
                nc.scalar.activation(tcc, c_sb, AF.Tanh)
                if t < T - 1:
                    # h-mul + 32x32-block transpose in column halves: PE
                    # K-round 0 only needs hT[:, 0:32], so it starts while
                    # the second half is still on the DVE.
                    hT = lhsT_pool.tile([128, 2 * 32], bf16, name="hT")
                    for hh in range(2):
                        cs = slice(32 * hh, 32 * hh + 32)
                        nc.vector.tensor_mul(h_sb[:, cs], sig_o[:, cs], tcc[:, cs])
                        nc.vector.transpose(out=hT[:, cs], in_=h_sb[:, cs])
                else:
                    # full-precision copy of the final h for the output
                    hf_sb = states.tile([128, S], fp32, name="hf_sb")
                    nc.vector.tensor_mul(hf_sb, sig_o, tcc)
                if u < TC - 1:
                    v = u + 1
                    g_ps = start_rounds(
                        xT_tiles[v // 8][:, 32 * (v % 8) : 32 * (v % 8) + 32]
                    )
                else:
                    g_ps = None  # reopened at the next chunk top
            if ch + 1 < n_chunks:
                x_cur = x_next
                xT_tiles = xT_next

        # ---- write back final h (unpack) ----
        for j in range(NJ):
            nc.sync.dma_start(
                out=hn_d[:, S * j : S * j + S], in_=hf_sb[32 * j : 32 * j + 32, :]
            )

    nc.compile()
    return nc


def _shard_inputs(x, h0, c0, w_ih, w_hh, b_ih, b_hh, T=T_FULL):
    import ml_dtypes

    bf16 = ml_dtypes.bfloat16
    (wih_fgi, whh_fgi, bias_fgi), (wih_o, whh_o, bias_o), ident = _prep_weights(
        np.asarray(w_ih, np.float32),
        np.asarray(w_hh, np.float32),
        np.asarray(b_ih, np.float32),
        np.asarray(b_hh, np.float32),
    )

    x = np.asarray(x, np.float32)
    h0 = np.asarray(h0, np.float32)
    c0 = np.asarray(c0, np.float32)
    common = {
        "wih_fgi": wih_fgi.astype(bf16),
        "wih_o": wih_o.astype(bf16),
        "whh_fgi": whh_fgi.astype(bf16),
        "whh_o": whh_o.astype(bf16),
        "b1_fgi": bias_fgi.astype(bf16),
        "b1_o": bias_o.astype(bf16),
        "ident": ident,
    }
    in_maps = []
    for k in range(NCORES):
        bs = slice(B * k, B * (k + 1))
        in_maps.append(
            {
                "x": np.ascontiguousarray(x[bs, :T, :]),
                "h0": np.ascontiguousarray(h0[0, bs, :]).astype(bf16),
                "c0": np.ascontiguousarray(c0[0, bs, :]),
                **common,
            }
        )
    return in_maps


_NC_CACHE = {}


def run_hw(x, h0, c0, w_ih, w_hh, b_ih, b_hh, T=T_FULL, TC=32, trace=False):
    _ensure_paths()
    from concourse.bass_utils import run_bass_kernel_spmd

    key = (T, TC)
    if key not in _NC_CACHE:
        _NC_CACHE[key] = build_nc(T=T, TC=TC)
    nc = _NC_CACHE[key]
    in_maps = _shard_inputs(x, h0, c0, w_ih, w_hh, b_ih, b_hh, T=T)
    res = run_bass_kernel_spmd(nc, in_maps, list(range(NCORES)), trace=trace)
    hn = np.stack([res.results[k]["hn"] for k in range(NCORES)], axis=0)
    return hn.reshape(1, B_TOT, H), res


def kernel(x, h0, c0, w_ih, w_hh, b_ih, b_hh):
    out, _ = run_hw(x, h0, c0, w_ih, w_hh, b_ih, b_hh)
    return out.astype(np.float32)


def _np_reference(x, h0, c0, w_ih, w_hh, b_ih, b_hh, T=None):
    """Numpy oracle for development (matches reference.py)."""
    x = np.asarray(x, np.float64)
    if T is not None:
        x = x[:, :T, :]
    h = np.asarray(h0, np.float64)[0]
    c = np.asarray(c0, np.float64)[0]
    gx = np.einsum("bti,gi->tbg", x, np.asarray(w_ih, np.float64)) + (
        np.asarray(b_ih, np.float64) + np.asarray(b_hh, np.float64)
    )
    W = np.asarray(w_hh, np.float64)

    def sg(v):
        return 1.0 / (1.0 + np.exp(-v))

    for t in range(x.shape[1]):
        g = gx[t] + h @ W.T
        i = sg(g[:, 0:256])
        f = sg(g[:, 256:512])
        gg = np.tanh(g[:, 512:768])
        o = sg(g[:, 768:1024])
        c = f * c + i * gg
        h = o * np.tanh(c)
    return h[None].astype(np.float32)


# revision 45
# speedup vs baseline: 1.2037x; 1.0091x over previous
"""LSTM (single layer, final hidden state) on 8 Trainium2 NeuronCores.

Reference computation (per batch row b):
    gx[t] = x[t] @ w_ih.T + (b_ih + b_hh)
    g     = gx[t] + h @ w_hh.T          # [B, 4H], gate order i,f,g,o
    i,f,o = sigmoid(...), g_c = tanh(...)
    c     = f*c + i*g_c
    h     = o * tanh(c)
returns h after T steps, shape [1, B, H].

Sharding: data-parallel over batch B=256 -> 8 cores x 32. Weights replicated.

Per-core layout ("packed"): partition p = 32*j + b, where j in [0,4) indexes
an H-quarter (H index = 64*j + s, s in [0,64)) and b in [0,32) is the local
batch.  All elementwise tiles are [128, *].

The wall time is T x the per-step dependency-chain latency.  On top of the
previous all-matmul/packing structure, this version crushes the ACT
(scalar-engine) spine cost: every ACT instruction costs ~N*0.83ns + ~150ns
fixed (SBUF/PSUM access pipe), so the 4 activation calls/step of the old
kernel (~1.4us of ACT busy, serialized on the spine) dominate.  Changes:
  * tanh(g) = 2*sigmoid(2g) - 1 with the 2x folded into the g rows of
    w_ih/w_hh/bias host-side -> ALL gates go through ONE Sigmoid call.
  * Gates grouped (f,g,i | o) in TWO psum tiles: sigmoid(f,g,i) [N=192]
    fires as soon as the fgi h-rounds stop (o-rounds still streaming);
    sigmoid(o) [N=64] runs off the c-spine in ACT's idle window.
  * c-chain on DVE: u=(2*sg-1)*si in ONE fused op (affine_mul_reduce),
    cf=f*c, c=cf+u; then tanh(c) [N=64] and h=o*tanh(c), hT transpose.
  * h-mul + 32x32-block transpose in column halves so the PE K-round 0
    starts while the second hT half is still on the DVE.
  * Gate tiles in fp16 (not bf16): kills the 2sg-1 cancellation error and
    halves DVE read traffic same as bf16; h stays bf16 for the PE lhsT.
  * c state in fp32 SBUF (DVE access 58cyc vs 120 psum).
  * Single bf16 bias round (abs err ~2e-4, damped by the f<1 recurrence).
  * sigma_fgi emitted BEFORE the o h-rounds: Tile coarsens sem-wait
    thresholds to the latest emitted producer-engine instruction, so
    emitting it later made sigma wait on the o rounds too.

Everything h-independent stays off the spine: x rounds for step t+1 opened
during step t's elementwise window, x chunks DMA-prefetched, PE-transposed
at the step top of 4 spread steps per chunk, and their psum->SBUF cast
rides the DVE idle window under tanh.

Measured (trace-driven facts for future iterations):
  * The PE never leaves the 4/8 HAM throttle in this environment (a 17us
    contiguous matmul burst stayed at the cold (219+N)/1.2 latency, N/1.2
    pipelined rate) -- model everything at 1.2GHz.
  * ACT instruction: ~(N+310)/1.2 ns; DVE op: ~(N+58)*1.04+45 ns with
    ~150ns issue-to-issue spacing; tensor_tensor_scan runs 2 cyc/elem
    (tried for the c update: net loss, reverted).
  * Engine wait-queues (depth 4) let ready instructions pass a parked one;
    a 400ns cast parked mid-FIFO displaces the h/hT tail on prep steps.

Measured on trn2 via axon: 2961488 ns (vs 3351814 ns for the previous
session's kernel re-measured in this environment), rel err 6.1e-3.
"""

import os
import sys

import numpy as np

B_TOT, T_FULL, I_DIM, H = 256, 1024, 128, 256
NCORES = 8
B = B_TOT // NCORES  # 32 per core
NJ = 4  # H quarters
S = H // NJ  # 64
# (row base in the PyTorch i,f,g,o layout, pre-scale) per column block
FGI_BLOCKS = ((256, 1.0), (512, 2.0), (0, 1.0))  # f, g (x2 for 2*sig(2g)-1), i
O_BLOCKS = ((768, 1.0),)
W_FGI = 3 * S  # 192
W_O = S  # 64


def _ensure_paths():
    for p in ("/opt/trn_rl_repo",):
        if os.path.isdir(p) and p not in sys.path:
            sys.path.append(p)


def _pack_tile(w_ih, w_hh, bsum, blocks):
    """Host-side permutation of weights into one gate-group tile layout."""
    ncol = S * len(blocks)
    wih = np.empty((I_DIM, NJ, ncol), np.float32)
    whh = np.empty((128, 2, NJ, ncol), np.float32)
    bias = np.empty((1, NJ, ncol), np.float32)
    # DVE 32x32 block-transpose of packed h puts H-input index
    # 64*(k//32) + 32*u + (k%32) at partition k of lhsT column-group u.
    k = np.arange(128)
    hperm = [64 * (k // 32) + 32 * u + (k % 32) for u in range(2)]
    for q, (rb, scale) in enumerate(blocks):
        for j in range(NJ):
            rows = slice(rb + S * j, rb + S * j + S)
            wih[:, j, S * q : S * q + S] = scale * w_ih[rows, :].T
            for u in range(2):
                whh[:, u, j, S * q : S * q + S] = (
                    scale * w_hh[rows, :][:, hperm[u]].T
                )
            bias[0, j, S * q : S * q + S] = scale * bsum[rows]
    return wih, whh, bias


def _prep_weights(w_ih, w_hh, b_ih, b_hh):
    bsum = (b_ih + b_hh).astype(np.float32)
    fgi = _pack_tile(w_ih, w_hh, bsum, FGI_BLOCKS)
    o = _pack_tile(w_ih, w_hh, bsum, O_BLOCKS)
    ident = np.zeros((128, 32), np.float32)
    for p in range(128):
        ident[p, p % 32] = 1.0
    return fgi, o, ident


def build_nc(T=T_FULL, TC=32, debug=False):
    """Build the per-core Bass program (SPMD: same program on all cores)."""
    _ensure_paths()
    import concourse.bacc as bacc
    import concourse.mybir as mybir
    import concourse.tile as tile
    from contextlib import ExitStack

    fp32 = mybir.dt.float32
    fp16 = mybir.dt.float16
    bf16 = mybir.dt.bfloat16
    AF = mybir.ActivationFunctionType
    ALU = mybir.AluOpType

    assert T % TC == 0 and TC % 8 == 0

    nc = bacc.Bacc("TRN2", target_bir_lowering=False, debug=debug)

    x_d = nc.dram_tensor("x", [B, T, I_DIM], fp32, kind="ExternalInput").ap()
    h0_d = nc.dram_tensor("h0", [B, H], bf16, kind="ExternalInput").ap()
    c0_d = nc.dram_tensor("c0", [B, H], fp32, kind="ExternalInput").ap()
    # x/h weights in bf16: matmuls stream at 1 cycle/row at any N and keep the
    # 4-way PE column-group concurrency (fp32 is 2 half-speed passes; fp32r
    # forbids dst partitions != 0, which the column groups need).
    wih_fgi_d = nc.dram_tensor(
        "wih_fgi", [I_DIM, NJ, W_FGI], bf16, kind="ExternalInput"
    ).ap()
    wih_o_d = nc.dram_tensor(
        "wih_o", [I_DIM, NJ, W_O], bf16, kind="ExternalInput"
    ).ap()
    whh_fgi_d = nc.dram_tensor(
        "whh_fgi", [128, 2, NJ, W_FGI], bf16, kind="ExternalInput"
    ).ap()
    whh_o_d = nc.dram_tensor(
        "whh_o", [128, 2, NJ, W_O], bf16, kind="ExternalInput"
    ).ap()
    # bias split b = b1 + b2 with b1 = bf16(b), b2 = bf16(b - b1): two bf16
    # K=1 rounds reproduce the fp32 bias to ~1e-6 while streaming single-pass.
    b_d = {}
    for nm, w in (("fgi", W_FGI), ("o", W_O)):
        b_d[nm, 1] = nc.dram_tensor(
            f"b1_{nm}", [1, NJ, w], bf16, kind="ExternalInput"
        ).ap()
    ident_d = nc.dram_tensor("ident", [128, 32], fp32, kind="ExternalInput").ap()
    hn_d = nc.dram_tensor("hn", [B, H], fp32, kind="ExternalOutput").ap()

    with tile.TileContext(nc) as tc, ExitStack() as ctx:
        consts = ctx.enter_context(tc.tile_pool(name="consts", bufs=1))
        states = ctx.enter_context(tc.tile_pool(name="states", bufs=1))
        # Deep rotations everywhere SBUF allows: Tile coarsens WAR sem
        # thresholds toward recent instructions, so shallow pools rate-limit
        # the steady state ~65ns/step above the spine latency (bufs 4->6 on
        # ew/lhsT measured -22us total; psum is capped by its 8 banks).
        lhsT_pool = ctx.enter_context(tc.tile_pool(name="lhsT", bufs=8))
        x_pool = ctx.enter_context(tc.tile_pool(name="xstream", bufs=3))
        xT_pool = ctx.enter_context(tc.tile_pool(name="xT", bufs=8))
        ew_pool = ctx.enter_context(tc.tile_pool(name="ew", bufs=8))
        # bufs=3: with 2, the next step's bias round inherits a WAR dep that
        # resolves only at the CURRENT step's last psum read, pushing it (cold)
        # into the critical window.
        # bufs=3 is the PSUM ceiling: tiles are bank-granular (fgi+o = 2
        # banks/step), 3 gens + 2 xt staging banks = all 8 banks.
        g_psum = ctx.enter_context(tc.tile_pool(name="g_psum", bufs=3, space="PSUM"))
        # bufs=2: with 1, the next prep group's PE transposes inherit a WAR
        # wait on the previous group's cast, which the scheduler then pushes
        # into the spine's h/hT tail on prep steps.
        xt_psum = ctx.enter_context(tc.tile_pool(name="xt_psum", bufs=2, space="PSUM"))

        # ---- constants ----
        wih_fgi_sb = consts.tile([I_DIM, NJ, W_FGI], bf16, name="wih_fgi_sb")
        nc.sync.dma_start(out=wih_fgi_sb, in_=wih_fgi_d)
        wih_o_sb = consts.tile([I_DIM, NJ, W_O], bf16, name="wih_o_sb")
        nc.sync.dma_start(out=wih_o_sb, in_=wih_o_d)
        whh_fgi_sb = consts.tile([128, 2, NJ, W_FGI], bf16, name="whh_fgi_sb")
        nc.sync.dma_start(out=whh_fgi_sb, in_=whh_fgi_d)
        whh_o_sb = consts.tile([128, 2, NJ, W_O], bf16, name="whh_o_sb")
        nc.sync.dma_start(out=whh_o_sb, in_=whh_o_d)
        b_sb = {}
        for nm, w in (("fgi", W_FGI), ("o", W_O)):
            t_ = consts.tile([1, NJ, w], bf16, name=f"b1_{nm}_sb")
            nc.sync.dma_start(out=t_, in_=b_d[nm, 1])
            b_sb[nm, 1] = t_
        ident_sb = consts.tile([128, 32], fp32, name="ident_sb")
        nc.sync.dma_start(out=ident_sb, in_=ident_d)
        ones_sb = consts.tile([1, 32], bf16, name="ones_sb")
        nc.vector.memset(ones_sb, 1.0)
        # NOTE: a 17us contiguous burst of back-to-back matmuls measured NO
        # HAM un-throttle on this platform -- the PE streams at 1.2GHz
        # permanently.  No warm-up tricks help; all cost models below assume
        # the cold (219+N)/1.2 latency and N/1.2 pipelined rate.

        # ---- state init (packed) ----
        c_sb = states.tile([128, S], fp32, name="c_sb")
        # h only feeds the gate matmuls (via the transpose), so it lives in
        # bf16; the final step writes a separate fp32 copy for the output.
        h_sb = states.tile([128, S], bf16, name="h_sb")
        for j in range(NJ):
            nc.sync.dma_start(
                out=c_sb[32 * j : 32 * j + 32, :], in_=c0_d[:, S * j : S * j + S]
            )
            nc.sync.dma_start(
                out=h_sb[32 * j : 32 * j + 32, :], in_=h0_d[:, S * j : S * j + S]
            )

        def emit_hT():
            """DVE 32x32 block transpose of packed h -> lhsT column groups."""
            hT = lhsT_pool.tile([128, 2 * 32], bf16, name="hT")
            nc.vector.transpose(out=hT, in_=h_sb)
            return hT

        hT = emit_hT()

        n_chunks = T // TC

        def fetch(ch):
            """Start the async HBM read of one x chunk (prefetched 1 ahead)."""
            x_sb = x_pool.tile([B, TC, I_DIM], fp32, name="x_sb")
            nc.sync.dma_start(out=x_sb, in_=x_d[:, ch * TC : (ch + 1) * TC, :])
            return x_sb

        def prep_pe(x_sb, g8):
            """PE-transpose 8 steps' x into a psum staging tile."""
            xt_ps = xt_psum.tile([128, 8 * 32], fp32, name="xt_ps")
            for v in range(8):
                nc.tensor.transpose(
                    out=xt_ps[:, 32 * v : 32 * v + 32],
                    in_=x_sb[:, g8 * 8 + v, :],
                    identity=ident_sb[0:32, :],
                    tile_position=(0, 0),
                )
            return xt_ps

        def prep_cast(xt_ps):
            xT_sb = xT_pool.tile([128, 8 * 32], bf16, name="xT_sb")
            nc.vector.tensor_copy(out=xT_sb, in_=xt_ps)
            return xT_sb

        def prep_group(x_sb, g8):
            return prep_cast(prep_pe(x_sb, g8))

        def start_rounds(xT_sl):
            """Open a step's psum accumulation: bias + x rounds (h-independent,
            so they run on the PE as soon as the bank frees, well before hT)."""
            g_fgi = g_psum.tile([128, W_FGI], fp32, name="g_fgi")
            g_o = g_psum.tile([128, W_O], fp32, name="g_o")
            for g_ps, nm, wsb in ((g_fgi, "fgi", wih_fgi_sb), (g_o, "o", wih_o_sb)):
                # Single bf16 bias round: abs err ~2e-4, constant every step,
                # damped by the f<1 recurrence -> ~1e-4 in h.  (The old
                # b1+b2 split cost 8 more K=1 matmuls per step.)
                for j in range(NJ):
                    nc.tensor.matmul(
                        g_ps[32 * j : 32 * j + 32, :],
                        ones_sb, b_sb[nm, 1][0:1, j, :],
                        start=True, stop=False,
                        tile_position=(0, 32 * j), skip_group_check=True,
                    )
                for j in range(NJ):
                    nc.tensor.matmul(
                        g_ps[32 * j : 32 * j + 32, :], xT_sl, wsb[:, j, :],
                        start=False, stop=False,
                        tile_position=(0, 32 * j), skip_group_check=True,
                    )
            return (g_fgi, g_o)

        x_cur = fetch(0)
        xT_tiles = [prep_group(x_cur, g8) for g8 in range(TC // 8)]
        g_ps = None
        for ch in range(n_chunks):
            if ch + 1 < n_chunks:
                x_next = fetch(ch + 1)
            xT_next = [None] * (TC // 8)
            if g_ps is None:
                g_ps = start_rounds(xT_tiles[0][:, 0:32])
            for u in range(TC):
                t = ch * TC + u
                g_fgi, g_o = g_ps
                # Next chunk's x-transposes at the step TOP: in the PE queue
                # they run during the PREVIOUS step's elementwise window
                # (ahead of the h-rounds, which wait on hT anyway).  Emitted
                # after add, they measured ~1us late and their CAST cascaded
                # into the h/transpose tail (+600ns on those steps).
                if u % 8 == 4 and ch + 1 < n_chunks:
                    # cast emitted right behind its producer transposes: its
                    # PE-counter wait then covers ONLY them (not this step's
                    # h/o rounds, where Tile's coarsened threshold previously
                    # landed), so it runs in the DVE idle gap before the
                    # c-chain instead of displacing the h/hT tail.
                    xT_next[(u - 4) // 8] = prep_cast(
                        prep_pe(x_next, (u - 4) // 8)
                    )
                # h rounds: the only h_{t-1}-dependent matmuls; fgi tile first
                # (and stopped first) so sigmoid(f,g,i) starts while the o
                # rounds still stream.  sigma_fgi is EMITTED before the o
                # rounds so its PE-counter wait threshold cannot get bumped
                # past them (Tile coarsens wait thresholds to the latest
                # emitted instruction on the producer engine).
                def h_rounds(g_half, whh_sb_):
                    for rnd in range(2):
                        for j in range(NJ):
                            nc.tensor.matmul(
                                g_half[32 * j : 32 * j + 32, :],
                                hT[:, 32 * rnd : 32 * rnd + 32],
                                whh_sb_[:, rnd, j, :],
                                start=False, stop=(rnd == 1),
                                tile_position=(0, 32 * j), skip_group_check=True,
                            )

                h_rounds(g_fgi, whh_fgi_sb)
                # gates: fgi cols [0:64]=f [64:128]=g' [128:192]=i; o tile.
                # fp16 gate tiles: DVE 2-byte fast path without bf16's 8-bit
                # mantissa (the 2sg-1 rewrite would cancel to ~4e-3 abs in
                # bf16; fp16 keeps it ~5e-4).
                sig = ew_pool.tile([128, W_FGI], fp16, name="sig")
                nc.scalar.activation(sig, g_fgi, AF.Sigmoid)
                h_rounds(g_o, whh_o_sb)
                sig_o = ew_pool.tile([128, S], bf16, name="sig_o")
                nc.scalar.activation(sig_o, g_o, AF.Sigmoid)
                # c-chain: c = f*c + (2*sg-1)*i; the (2sg-1)*i product is ONE
                # fused DVE op (affine_mul_reduce: (in0*2-1)*in1).
                uu = ew_pool.tile([128, S], bf16, name="uu")
                uacc = ew_pool.tile([128, 1], fp32, name="uacc")
                nc.vector.affine_mul_reduce(
                    out=uu, accum_out=uacc,
                    in0=sig[:, S : 2 * S], in1=sig[:, 2 * S : 3 * S],
                    scale=2.0, bias=-1.0,
                )
                cf = ew_pool.tile([128, S], fp32, name="cf")
                nc.vector.tensor_mul(cf, sig[:, 0:S], c_sb)
                nc.vector.tensor_add(c_sb, cf, uu)
                tcc = ew_pool.tile([128, S], bf16, name="tcc")
                nc.scalar.activation(tcc, c_sb, AF.Tanh)
                if t < T - 1:
                    # h-mul + 32x32-block transpose in column halves: PE
                    # K-round 0 only needs hT[:, 0:32], so it starts while
                    # the second half is still on the DVE.
                    hT = lhsT_pool.tile([128, 2 * 32], bf16, name="hT")
                    for hh in range(2):
                        cs = slice(32 * hh, 32 * hh + 32)
                        nc.vector.tensor_mul(h_sb[:, cs], sig_o[:, cs], tcc[:, cs])
                        nc.vector.transpose(out=hT[:, cs], in_=h_sb[:, cs])
                else:
                    # full-precision copy of the final h for the output
                    hf_sb = states.tile([128, S], fp32, name="hf_sb")
                    nc.vector.tensor_mul(hf_sb, sig_o, tcc)
                if u < TC - 1:
                    v = u + 1
                    g_ps = start_rounds(
                        xT_tiles[v // 8][:, 32 * (v % 8) : 32 * (v % 8) + 32]
                    )
                else:
                    g_ps = None  # reopened at the next chunk top
            if ch + 1 < n_chunks:
                x_cur = x_next
                xT_tiles = xT_next

        # ---- write back final h (unpack) ----
        for j in range(NJ):
            nc.sync.dma_start(
                out=hn_d[:, S * j : S * j + S], in_=hf_sb[32 * j : 32 * j + 32, :]
            )

    nc.compile()
    return nc


def _shard_inputs(x, h0, c0, w_ih, w_hh, b_ih, b_hh, T=T_FULL):
    import ml_dtypes

    bf16 = ml_dtypes.bfloat16
    (wih_fgi, whh_fgi, bias_fgi), (wih_o, whh_o, bias_o), ident = _prep_weights(
        np.asarray(w_ih, np.float32),
        np.asarray(w_hh, np.float32),
        np.asarray(b_ih, np.float32),
        np.asarray(b_hh, np.float32),
    )

    x = np.asarray(x, np.float32)
    h0 = np.asarray(h0, np.float32)
    c0 = np.asarray(c0, np.float32)
    common = {
        "wih_fgi": wih_fgi.astype(bf16),
        "wih_o": wih_o.astype(bf16),
        "whh_fgi": whh_fgi.astype(bf16),
        "whh_o": whh_o.astype(bf16),
        "b1_fgi": bias_fgi.astype(bf16),
        "b1_o": bias_o.astype(bf16),
        "ident": ident,
    }
    in_maps = []
    for k in range(NCORES):
        bs = slice(B * k, B * (k + 1))
        in_maps.append(
            {
                "x": np.ascontiguousarray(x[bs, :T, :]),
                "h0": np.ascontiguousarray(h0[0, bs, :]).astype(bf16),
                "c0": np.ascontiguousarray(c0[0, bs, :]),
                **common,
            }
        )
    return in_maps


_NC_CACHE = {}


def run_hw(x, h0, c0, w_ih, w_hh, b_ih, b_hh, T=T_FULL, TC=32, trace=False):
    _ensure_paths()
    from concourse.bass_utils import run_bass_kernel_spmd

    key = (T, TC)
    if key not in _NC_CACHE:
        _NC_CACHE[key] = build_nc(T=T, TC=TC)
    nc = _NC_CACHE[key]
    in_maps = _shard_inputs(x, h0, c0, w_ih, w_hh, b_ih, b_hh, T=T)
    res = run_bass_kernel_spmd(nc, in_maps, list(range(NCORES)), trace=trace)
    hn = np.stack([res.results[k]["hn"] for k in range(NCORES)], axis=0)
    return hn.reshape(1, B_TOT, H), res


def kernel(x, h0, c0, w_ih, w_hh, b_ih, b_hh):
    out, _ = run_hw(x, h0, c0, w_ih, w_hh, b_ih, b_hh)
    return out.astype(np.float32)


def _np_reference(x, h0, c0, w_ih, w_hh, b_ih, b_hh, T=None):
    """Numpy oracle for development (matches reference.py)."""
    x = np.asarray(x, np.float64)
    if T is not None:
        x = x[:, :T, :]
    h = np.asarray(h0, np.float64)[0]
    c = np.asarray(c0, np.float64)[0]
    gx = np.einsum("bti,gi->tbg", x, np.asarray(w_ih, np.float64)) + (
        np.asarray(b_ih, np.float64) + np.asarray(b_hh, np.float64)
    )
    W = np.asarray(w_hh, np.float64)

    def sg(v):
        return 1.0 / (1.0 + np.exp(-v))

    for t in range(x.shape[1]):
        g = gx[t] + h @ W.T
        i = sg(g[:, 0:256])
        f = sg(g[:, 256:512])
        gg = np.tanh(g[:, 512:768])
        o = sg(g[:, 768:1024])
        c = f * c + i * gg
        h = o * np.tanh(c)
    return h[None].astype(np.float32)
